# revision 18
# baseline (speedup 1.0000x reference)
"""HF OpenMoe attention (B=2,S=2048,HID=2048,NH=16,NKV=4,HD=128) on 8 trn2 cores.

Sharding: core c -> (batch b=c//4, kv-group g=c%4). Each core computes Q/K/V
projections for its 4 query heads + 1 kv head, RoPE, causal flash attention in
S^T layout (scores transposed: [k, q], softmax over the partition dim via
ones-matmul), and its partial o_proj; a 4-way ReduceScatter sums the o_proj
partials, each core returning a 512-row slice of o^T for its batch.

Schedule: phase A streams x^T in 4-ci batched DMAs with weights interleaved
earliest-needed-first (wk halves -> wq heads -> wv), emitting each projection
chain as soon as its operands land; V transposes of block b interleave into
block b+1's Q heads. Phase B runs attention per q-block in two 2-head passes:
score pairs land in a 2-bank PSUM tile, one exp covers both heads, mask/acc
run as paired 2x DVE ops, and o_proj of the previous block is emitted in
1-co units between score tiles so the PE never waits on the Act-paced exp
stream. The final block (j=0, shortest) keeps a few reserve units to cover
its normalize latency, and the tail o_proj DMAs are split small.
"""
import numpy as np
import concourse.bass as bass
import concourse.bacc as bacc
import concourse.tile as tile
import concourse.mybir as mybir
from concourse.bass_utils import run_bass_kernel_spmd

f32 = mybir.dt.float32
f32r = mybir.dt.float32r
bf16 = mybir.dt.bfloat16
AF = mybir.ActivationFunctionType
MUL = mybir.AluOpType.mult
ADD = mybir.AluOpType.add

B, S, HID = 2, 2048, 2048
NH, NKV, HD = 16, 4, 128
GH = NH // NKV          # query heads per core (4)
TB = 512                # token block (q block / projection block)
NT = S // TB            # 4 token blocks
NCT = HID // 128        # 16 contraction tiles
NKT = S // 128          # 16 key tiles

_CACHE = {}

def _build(causal: bool, with_rs: bool = True):
    nc = bacc.Bacc("TRN2", target_bir_lowering=False, debug=False, num_devices=8)
    xt = nc.dram_tensor("xt", [HID, S], f32, kind="ExternalInput").ap()
    wq = nc.dram_tensor("wq", [HID, GH * HD], f32, kind="ExternalInput").ap()
    wk = nc.dram_tensor("wk", [HID, HD], f32, kind="ExternalInput").ap()
    wv = nc.dram_tensor("wv", [HID, HD], f32, kind="ExternalInput").ap()
    wo = nc.dram_tensor("wo", [GH * HD, HID], f32, kind="ExternalInput").ap()
    tab_d = nc.dram_tensor("ropetab", [128, NT * 2 * TB], f32,
                           kind="ExternalInput").ap()
    cm_d = nc.dram_tensor("cmask", [128, 1536], bf16, kind="ExternalInput").ap()
    on_d = nc.dram_tensor("ones_in", [128, 128], bf16, kind="ExternalInput").ap()
    id_d = nc.dram_tensor("ident_in", [128, 128], bf16, kind="ExternalInput").ap()
    out_r = nc.dram_tensor("out_r", [TB, S], f32, kind="ExternalOutput").ap()

    with tile.TileContext(nc) as tc:
        with (
            tc.tile_pool(name="glob", bufs=1) as glob,
            tc.tile_pool(name="dram", bufs=1, space="DRAM") as dram,
        ):
            # ---- global resident stores ----
            kt_rope = glob.tile([128, S], f32r, tag="kt")          # roped K^T [d, k]
            v_all = glob.tile([128, S], bf16, tag="v")             # V natural (bf16)
            qt_rope = [glob.tile([128, S], f32r, tag=f"q{h}", name=f"qt_rope{h}")
                       for h in range(GH)]
            cm_b = glob.tile([128, 1536], bf16, tag="cmb")         # paired 0/1 masks
            ones_b = glob.tile([128, 128], bf16, tag="onesb")
            ident_b = glob.tile([128, 128], bf16, tag="identb")
            wo_all = glob.tile([128, GH * HID], f32r, tag="wo")    # [j-sub, jh*2048+c]

            oT_part = dram.tile([HID, S], f32)                     # o^T partial
            oT_red = dram.tile([TB, S], f32)

            # ---- phase A: projections + rope (phase-scoped SBUF) ----
            with tc.tile_pool(name="pA", bufs=1) as pA, \
                 tc.tile_pool(name="psA", bufs=1, space="PSUM") as psA:
                wk_all = pA.tile([128, NCT, HD], f32r, tag="wk")
                wv_all = pA.tile([128, NCT, HD], f32r, tag="wv")
                wq_all = [pA.tile([128, NCT, HD], f32r, tag=f"wqh{h}",
                                  name=f"wq_all{h}") for h in range(GH)]
                xt_t = {}    # (tb, u) -> [128, 4, TB] tile holding ci 4u..4u+3
                tabs = {}

                def load_xt(tb, u, half=None):
                    if (tb, u) not in xt_t:
                        xt_t[(tb, u)] = pA.tile([128, 4, TB], f32r, tag="xt",
                                                bufs=6, name="xt")
                    t = xt_t[(tb, u)]
                    lo, n = (0, 4) if half is None else (2 * half, 2)
                    nc.sync.dma_start(
                        t[:, lo:lo + n, :],
                        xt[512 * u + 128 * lo:512 * u + 128 * (lo + n),
                           TB * tb:TB * (tb + 1)].bitcast(f32r)
                        .rearrange("(c p) t -> p c t", p=128))

                def load_w(dst, src, lo=0, n=NCT):
                    nc.sync.dma_start(
                        dst[:, lo:lo + n, :],
                        src[128 * lo:128 * (lo + n), :].bitcast(f32r)
                        .rearrange("(c p) d -> p c d", p=128))

                def load_tab(tb):
                    tabs[tb] = pA.tile([128, 2, TB], f32, tag="tab", bufs=2,
                                       name="tab")
                    nc.sync.dma_start(
                        tabs[tb][:],
                        tab_d[:, 2 * TB * tb:2 * TB * (tb + 1)]
                        .rearrange("p (u t) -> p u t", u=2))

                # DMA emission order (sync queue drains in order): fine-grained
                # first chunks so the PE starts ASAP, then weights interleaved
                # earliest-needed-first, then the xt stream with rope tables
                # just ahead of each block.
                nc.sync.dma_start(ident_b[:], id_d[:])
                load_w(wk_all, wk, 0, 8)
                load_xt(0, 0, half=0)
                load_w(wk_all, wk, 8, 8)
                load_xt(0, 0, half=1)
                load_tab(0)
                load_w(wq_all[0], wq[:, 0:HD])
                load_xt(0, 1)
                load_w(wq_all[1], wq[:, HD:2 * HD])
                load_xt(0, 2)
                load_w(wq_all[2], wq[:, 2 * HD:3 * HD])
                load_xt(0, 3)
                load_w(wq_all[3], wq[:, 3 * HD:4 * HD])
                load_w(wv_all, wv)
                nc.sync.dma_start(ones_b[:], on_d[:])
                nc.sync.dma_start(cm_b[:], cm_d[:])
                for tb in range(1, NT):
                    load_xt(tb, 0)
                    load_tab(tb)
                    for u in range(1, 4):
                        load_xt(tb, u)
                # wo lands after the xt stream (~late phase A), well before the
                # first o_proj unit in phase B
                nc.sync.dma_start(wo_all[:].rearrange("p (h c) -> p h c", h=GH),
                                  wo[:].bitcast(f32r)
                                  .rearrange("(h p) c -> p h c", p=128))

                def rope(ps, dst_ap, tb):
                    """dst = ps*cos + swap64(ps)*sin_mod for token block tb."""
                    cs, sn = tabs[tb][:, 0, :], tabs[tb][:, 1, :]
                    raw = pA.tile([128, TB], f32, tag="raw", bufs=2, name="raw")
                    nc.scalar.copy(raw[:], ps[:])
                    rot = pA.tile([128, TB], f32, tag="rot", bufs=4, name="rot")
                    nc.gpsimd.dma_start(rot[0:64, :], raw[64:128, :])
                    nc.gpsimd.dma_start(rot[64:128, :], raw[0:64, :])
                    m1 = pA.tile([128, TB], f32, tag="m1", bufs=4, name="m1")
                    nc.vector.tensor_tensor(m1[:], ps[:], cs, op=MUL)
                    nc.vector.tensor_tensor(rot[:], rot[:], sn, op=MUL)
                    nc.vector.tensor_tensor(dst_ap, m1[:], rot[:], op=ADD)

                def mm_chain(ps, wt, tb, lo, n):
                    for ci in range(lo, lo + n):
                        nc.tensor.matmul(ps[:], wt[:, ci, :],
                                         xt_t[(tb, ci // 4)][:, ci % 4, :],
                                         start=ci == 0, stop=ci == NCT - 1)

                vt_pend = []   # deferred V-transpose units (tb, vt_sb, u)

                def emit_vtrans():
                    if not vt_pend:
                        return
                    tb, vt_sb, u = vt_pend.pop(0)
                    ps_tr = psA.tile([128, 128], bf16, tag="ptr", bufs=1,
                                     name="ps_tr")
                    nc.tensor.transpose(ps_tr[:],
                                        vt_sb[:, 128 * u:128 * (u + 1)],
                                        ident_b[:])
                    nc.scalar.copy(
                        v_all[:, 128 * (4 * tb + u):128 * (4 * tb + u + 1)],
                        ps_tr[:])

                for tb in range(NT):
                    ps_k = psA.tile([128, TB], f32, tag="pk")
                    ps_q = [psA.tile([128, TB], f32, tag="pq", bufs=5,
                                     name=f"ps_q{h}") for h in range(GH)]
                    if tb == 0:
                        # interleaved with DMA arrivals: K/Q chains advance as
                        # their xt batches + weight tiles land
                        mm_chain(ps_k, wk_all, 0, 0, 4)
                        warm = pA.tile([128, 1], f32, tag="warm")
                        nc.scalar.activation(warm[:], ident_b[:, 0:1], AF.Exp)
                        mm_chain(ps_q[0], wq_all[0], 0, 0, 4)
                        mm_chain(ps_k, wk_all, 0, 4, 4)
                        mm_chain(ps_q[0], wq_all[0], 0, 4, 4)
                        mm_chain(ps_q[1], wq_all[1], 0, 0, 8)
                        mm_chain(ps_k, wk_all, 0, 8, 4)
                        mm_chain(ps_q[0], wq_all[0], 0, 8, 4)
                        mm_chain(ps_q[1], wq_all[1], 0, 8, 4)
                        mm_chain(ps_q[2], wq_all[2], 0, 0, 12)
                        mm_chain(ps_k, wk_all, 0, 12, 4)
                        rope(ps_k, kt_rope[:, 0:TB], 0)
                        mm_chain(ps_q[0], wq_all[0], 0, 12, 4)
                        rope(ps_q[0], qt_rope[0][:, 0:TB], 0)
                        mm_chain(ps_q[1], wq_all[1], 0, 12, 4)
                        rope(ps_q[1], qt_rope[1][:, 0:TB], 0)
                        mm_chain(ps_q[2], wq_all[2], 0, 12, 4)
                        rope(ps_q[2], qt_rope[2][:, 0:TB], 0)
                        mm_chain(ps_q[3], wq_all[3], 0, 0, 16)
                        rope(ps_q[3], qt_rope[3][:, 0:TB], 0)
                    else:
                        mm_chain(ps_k, wk_all, tb, 0, NCT)
                        rope(ps_k, kt_rope[:, TB * tb:TB * (tb + 1)], tb)
                        # V right after K so the xt ring slots release early
                        # (the in-order DMA queue must not wait on them)
                        ps_v = psA.tile([128, TB], f32, tag="pv")
                        mm_chain(ps_v, wv_all, tb, 0, NCT)
                        vt_sb = pA.tile([128, TB], bf16, tag="vts", bufs=2,
                                        name="vt_sb")
                        nc.scalar.copy(vt_sb[:], ps_v[:])
                        vt_pend.extend((tb, vt_sb, u) for u in range(4))
                        for h in range(GH):
                            emit_vtrans()
                            if len(vt_pend) > GH - 1 - h:
                                emit_vtrans()
                            mm_chain(ps_q[h], wq_all[h], tb, 0, NCT)
                            rope(ps_q[h], qt_rope[h][:, TB * tb:TB * (tb + 1)],
                                 tb)
                    if tb == 0:
                        # block 0's V runs after its Q heads (wv arrives last);
                        # its transposes interleave into block 1's Q stream
                        ps_v = psA.tile([128, TB], f32, tag="pv")
                        mm_chain(ps_v, wv_all, 0, 0, NCT)
                        vt_sb = pA.tile([128, TB], bf16, tag="vts", bufs=2,
                                        name="vt_sb")
                        nc.scalar.copy(vt_sb[:], ps_v[:])
                        vt_pend.extend((0, vt_sb, u) for u in range(4))
                while vt_pend:
                    emit_vtrans()

            # ---- phase B: attention (2-head passes) + partial o_proj ----
            with tc.tile_pool(name="pB", bufs=1) as pB, \
                 tc.tile_pool(name="psB", bufs=1, space="PSUM") as psB:

                units = []   # pending o_proj 1-co unit closures

                def oproj_unit(j, at_j, co, dma_n):
                    """One 128-col chunk of q-block j's o_proj. dma_n: if set,
                    flush this ob group (dma_n co tiles) to DRAM."""
                    g, gi = co // 2, co % 2
                    ob = oproj_unit.ob
                    if gi == 0:
                        ob = oproj_unit.ob = pB.tile([128, 2, TB], f32,
                                                     tag="ob", bufs=3,
                                                     name="ob")
                    ps_p = psB.tile([128, TB], f32, tag="ps_d", bufs=2,
                                    name="ps_p")
                    for jh in range(GH):
                        nc.tensor.matmul(ps_p[:],
                                         wo_all[:, jh * HID + 128 * co:
                                                jh * HID + 128 * (co + 1)],
                                         at_j[jh][:], start=(jh == 0),
                                         stop=(jh == GH - 1))
                    if co % 2 == 1:
                        nc.scalar.copy(ob[:, gi, :], ps_p[:])
                    else:
                        nc.vector.tensor_copy(ob[:, gi, :], ps_p[:])
                    if dma_n:
                        if co < 4 and not with_rs:
                            dst = out_r[128 * (co + 1 - dma_n):128 * (co + 1),
                                        TB * j:TB * (j + 1)]
                        else:
                            dst = oT_part[128 * (co + 1 - dma_n):
                                          128 * (co + 1), TB * j:TB * (j + 1)]
                        nc.scalar.dma_start(
                            dst.rearrange("(u p) t -> p u t", p=128),
                            ob[:, gi + 1 - dma_n:gi + 1, :])
                oproj_unit.ob = None

                def make_units(j, at_j, tail=False):
                    # groups of 2 co per DMA; on the tail block split the last
                    # two groups into single-co DMAs so the final transfer is
                    # short
                    out = []
                    for co in range(NCT):
                        if tail and co >= NCT - 4:
                            dma_n = 1
                        else:
                            dma_n = 2 if co % 2 == 1 else 0
                        out.append((lambda jj, aa, cc, dd:
                                    lambda: oproj_unit(jj, aa, cc, dd))
                                   (j, at_j, co, dma_n))
                    return out

                order = [1, 2, 3, 0] if causal else [1, 2, 3, 0]
                for bi, j in enumerate(order):
                    last = bi == len(order) - 1
                    if causal:
                        tiles = [(i, 0) for i in range(4 * j)]
                        tiles += [(4 * j + m, min(128 * m, 256))
                                  for m in range(4)]
                    else:
                        tiles = [(i, 0) for i in range(NKT)]
                    last_i = tiles[-1][0]
                    n_iter = 2 * len(tiles)
                    # interleave cadence: previous block's units spread over
                    # this block's tile stream; on the last block hold 4 units
                    # back to cover the final normalize latency
                    reserve = 4 if last else 0
                    spread = max(0, len(units) - reserve)
                    credit, it = 0.0, 0

                    at_j = [pB.tile([128, TB], f32r, tag=f"at{h}", bufs=2,
                                    name=f"at_s{h}") for h in range(GH)]
                    for p in range(2):
                        h0, h1 = 2 * p, 2 * p + 1
                        acc = pB.tile([128, 2 * TB], bf16, tag="accp", bufs=2,
                                      name="acc")
                        ps_o = {h: psB.tile([128, TB], f32, tag="po", bufs=2,
                                            name=f"ps_o{h}") for h in (h0, h1)}
                        for ti, (i, off) in enumerate(tiles):
                            w = TB - off
                            diag = causal and i >= 4 * j
                            m = i - 4 * j if diag else -1
                            ps2 = psB.tile([128, 2 * TB], f32, tag="ps_s",
                                           bufs=2, name="ps2")
                            for hh, h in enumerate((h0, h1)):
                                nc.tensor.matmul(
                                    ps2[:, TB * hh:TB * hh + w],
                                    kt_rope[:, 128 * i:128 * (i + 1)],
                                    qt_rope[h][:, TB * j + off:TB * (j + 1)],
                                    start=True, stop=True)
                            pt2 = pB.tile([128, 2 * TB], bf16, tag="pt",
                                          bufs=8, name="pt")
                            if w == TB:
                                nc.scalar.activation(pt2[:], ps2[:], AF.Exp)
                            else:
                                pr = pt2[:].rearrange("p (u q) -> p u q", u=2)
                                sr = ps2[:].rearrange("p (u q) -> p u q", u=2)
                                nc.scalar.activation(pr[:, :, 0:w],
                                                     sr[:, :, 0:w], AF.Exp)
                            if diag:
                                patt, pw = (1024, 256) if m == 3 else (0, TB)
                                cr = (cm_b[:, patt:patt + 2 * pw]
                                      .rearrange("p (u q) -> p u q", u=2))
                                pr = pt2[:].rearrange("p (u q) -> p u q", u=2)
                                nc.vector.tensor_tensor(
                                    pr[:, :, 0:w], pr[:, :, 0:w],
                                    cr[:, :, 0:w], op=MUL)
                            if ti == 0:
                                nc.vector.tensor_copy(acc[:], pt2[:])
                            else:
                                ar = acc[:].rearrange("p (u q) -> p u q", u=2)
                                pr = pt2[:].rearrange("p (u q) -> p u q", u=2)
                                nc.vector.tensor_tensor(
                                    ar[:, :, off:TB], ar[:, :, off:TB],
                                    pr[:, :, 0:w], op=ADD)
                            # PV (m=3 keep region is only the last 128 cols)
                            pv_off = 384 if m == 3 else off
                            for hh, h in enumerate((h0, h1)):
                                nc.tensor.matmul(
                                    ps_o[h][:, pv_off:TB],
                                    v_all[:, 128 * i:128 * (i + 1)],
                                    pt2[:, TB * hh + pv_off - off:
                                        TB * hh + TB - off],
                                    start=(ti == 0), stop=(i == last_i),
                                    skip_group_check=True)
                            it += 1
                            credit += spread / n_iter
                            while credit >= 1.0 and units:
                                credit -= 1.0
                                units.pop(0)()
                        # pass end: paired denominator into the ps_s ring (so
                        # the ps_d ring stays free for o_proj units), then a
                        # couple of units cover the po-bank WAR into the next
                        # pass while the reciprocal + normalize chain runs
                        pdp = psB.tile([128, 2 * TB], f32, tag="ps_s", bufs=2,
                                       name="pdp")
                        for hh in range(2):
                            nc.tensor.matmul(pdp[:, TB * hh:TB * (hh + 1)],
                                             ones_b[:],
                                             acc[:, TB * hh:TB * (hh + 1)],
                                             start=True, stop=True)
                        if len(units) >= 2:
                            units.pop(0)(); units.pop(0)()
                        rec = pB.tile([128, 2 * TB], f32, tag="rec", bufs=2,
                                      name="rec")
                        nc.vector.reciprocal(rec[:], pdp[:])
                        for hh, h in enumerate((h0, h1)):
                            nc.vector.tensor_tensor(
                                at_j[h][:], ps_o[h][:],
                                rec[:, TB * hh:TB * (hh + 1)], op=MUL)
                    while units:
                        units.pop(0)()
                    units = make_units(j, at_j, tail=last)
                while units:
                    units.pop(0)()

            # ---- phase C: ReduceScatter partials, emit this core's slice ----
            if with_rs:
                nc.gpsimd.collective_compute(
                    "ReduceScatter", ADD,
                    replica_groups=[[0, 1, 2, 3], [4, 5, 6, 7]],
                    ins=[oT_part[:].opt()], outs=[oT_red[:].opt()],
                )
                nc.sync.dma_start(out_r[:], oT_red[:])

    nc.compile()
    return nc


def kernel(hidden_states, attention_mask, Wq, Wk, Wv, Wo, sin, cos):
    hidden_states = np.asarray(hidden_states, dtype=np.float32)
    attention_mask = np.asarray(attention_mask, dtype=np.float32)
    Wq, Wk, Wv, Wo = (np.ascontiguousarray(np.asarray(a, dtype=np.float32))
                      for a in (Wq, Wk, Wv, Wo))
    sin = np.asarray(sin, dtype=np.float32)
    cos = np.asarray(cos, dtype=np.float32)

    # classify the mask: causal (top-right strictly very-negative, elsewhere 0,
    # col 0 ignored since reference zeroes it) vs all-zeros (full attention)
    m0 = attention_mask[0, 0]
    iu = np.triu_indices(S, k=1)
    causal = bool((m0[iu] < -1e30).all() and
                  (m0[np.tril_indices(S, k=0)] == 0.0).all())
    if not causal:
        assert (attention_mask == 0).all(), "unsupported attention mask pattern"
    if causal:
        for b in range(1, B):
            assert np.array_equal(attention_mask[b, 0], m0), "mask differs per batch"

    key = causal
    if key not in _CACHE:
        _CACHE[key] = _build(causal)
    nc = _CACHE[key]

    cos_t = np.ascontiguousarray(cos[:S].T)          # [128, S]
    sin_m = np.ascontiguousarray(sin[:S].T)
    sin_m[:64] *= -1.0
    # packed rope tables: per block [cos | sin_mod]
    tab = np.empty((128, NT, 2, TB), dtype=np.float32)
    for tb in range(NT):
        tab[:, tb, 0, :] = cos_t[:, TB * tb:TB * (tb + 1)]
        tab[:, tb, 1, :] = sin_m[:, TB * tb:TB * (tb + 1)]
    tab = np.ascontiguousarray(tab.reshape(128, NT * 2 * TB))
    # paired 0/1 causal keep-patterns (each repeated twice for head pairs):
    # patt0 = (q >= k) at cols 0:1024, patt1 = (q >= k + 128) at cols 1024:1536
    kl = np.arange(128)[:, None]
    ql = np.arange(512)[None, :]
    p0 = (ql >= kl).astype(np.float32)
    p1 = (ql[:, :256] >= kl + 128).astype(np.float32)
    cmask = np.concatenate([p0, p0, p1, p1], axis=1)

    in_maps = []
    for c in range(8):
        b, g = c // 4, c % 4
        in_maps.append({
            "xt": np.ascontiguousarray(hidden_states[b].T),
            "wq": np.ascontiguousarray(Wq[512 * g:512 * (g + 1), :].T),
            "wk": np.ascontiguousarray(Wk[128 * g:128 * (g + 1), :].T),
            "wv": np.ascontiguousarray(Wv[128 * g:128 * (g + 1), :].T),
            "wo": np.ascontiguousarray(Wo[:, 512 * g:512 * (g + 1)].T),
            "ropetab": tab, "cmask": cmask,
            "ones_in": np.ones((128, 128), dtype=np.float32),
            "ident_in": np.eye(128, dtype=np.float32),
        })

    global _LAST_IN_MAPS, _LAST_RES
    _LAST_IN_MAPS = in_maps
    res = run_bass_kernel_spmd(nc, in_maps, core_ids=list(range(8)))
    _LAST_RES = res

    out = np.empty((B, S, HID), dtype=np.float32)
    for c in range(8):
        b, r = c // 4, c % 4
        out[b, :, TB * r:TB * (r + 1)] = res.results[c]["out_r"].T
    return out


if __name__ == "__main__":
    print("module loads ok")


# revision 20
# speedup vs baseline: 1.1655x; 1.1655x over previous
"""HF OpenMoe attention (B=2,S=2048,HID=2048,NH=16,NKV=4,HD=128) on 8 trn2 cores.

Sharding: core c -> (batch b=c//4, kv-group g=c%4). Each core computes Q/K/V
projections for its 4 query heads + 1 kv head, RoPE, causal flash attention in
S^T layout (scores transposed: [k, q], softmax over the partition dim via
ones-matmul), and its partial o_proj; a 4-way ReduceScatter sums the o_proj
partials, each core returning a 512-row slice of o^T for its batch.

Phase A streams x^T per token block with weights interleaved into the DMA
stream (wk chunked between xt tiles, wv mid-stream, wq after block 0), PE
chains emitted as operands land, rope on DVE/Act/Pool. Phase B runs causal
attention per q-block in two 2-head passes: score pairs land in a 2-bank PSUM
tile, one exp covers both heads, mask/acc run as paired 2x DVE ops, the
paired denominator reuses the score-PSUM ring, and o_proj of the previous
block is emitted in 1-co units between score tiles so the PE keeps running
through the Act-paced exp stream. The final block (j=0, shortest) holds a few
units in reserve to cover its normalize latency; tail o_proj DMAs are split
small and output DMAs ride the Act queue.
"""
import numpy as np
import concourse.bass as bass
import concourse.bacc as bacc
import concourse.tile as tile
import concourse.mybir as mybir
from concourse.bass_utils import run_bass_kernel_spmd

f32 = mybir.dt.float32
f32r = mybir.dt.float32r
bf16 = mybir.dt.bfloat16
AF = mybir.ActivationFunctionType
MUL = mybir.AluOpType.mult
ADD = mybir.AluOpType.add

B, S, HID = 2, 2048, 2048
NH, NKV, HD = 16, 4, 128
GH = NH // NKV          # query heads per core (4)
TB = 512                # token block (q block / projection block)
NT = S // TB            # 4 token blocks
NCT = HID // 128        # 16 contraction tiles
NKT = S // 128          # 16 key tiles

_CACHE = {}


def _build(causal: bool, with_rs: bool = True):
    nc = bacc.Bacc("TRN2", target_bir_lowering=False, debug=False, num_devices=8)
    xt = nc.dram_tensor("xt", [HID, S], f32, kind="ExternalInput").ap()
    wq = nc.dram_tensor("wq", [HID, GH * HD], f32, kind="ExternalInput").ap()
    wk = nc.dram_tensor("wk", [HID, HD], f32, kind="ExternalInput").ap()
    wv = nc.dram_tensor("wv", [HID, HD], f32, kind="ExternalInput").ap()
    wo = nc.dram_tensor("wo", [GH * HD, HID], f32, kind="ExternalInput").ap()
    cos_d = nc.dram_tensor("cos_t", [HD, S], f32, kind="ExternalInput").ap()
    sin_d = nc.dram_tensor("sin_m", [HD, S], f32, kind="ExternalInput").ap()
    cm_d = nc.dram_tensor("cmask", [128, 1536], bf16, kind="ExternalInput").ap()
    on_d = nc.dram_tensor("ones_in", [128, 128], bf16, kind="ExternalInput").ap()
    id_d = nc.dram_tensor("ident_in", [128, 128], bf16, kind="ExternalInput").ap()
    out_r = nc.dram_tensor("out_r", [TB, S], f32, kind="ExternalOutput").ap()

    with tile.TileContext(nc) as tc:
        with (
            tc.tile_pool(name="glob", bufs=1) as glob,
            tc.tile_pool(name="dram", bufs=1, space="DRAM") as dram,
        ):
            # ---- global resident stores ----
            kt_rope = glob.tile([128, S], f32r, tag="kt")          # roped K^T [d, k]
            v_all = glob.tile([128, S], bf16, tag="v")             # V natural (bf16)
            qt_rope = [glob.tile([128, S], f32r, tag=f"q{h}", name=f"qt_rope{h}")
                       for h in range(GH)]
            cm_b = glob.tile([128, 1536], bf16, tag="cmb")         # paired 0/1 masks
            ones_b = glob.tile([128, 128], bf16, tag="onesb")
            ident_b = glob.tile([128, 128], bf16, tag="identb")

            oT_part = dram.tile([HID, S], f32)                     # o^T partial
            oT_red = dram.tile([TB, S], f32)

            # ---- phase A: projections + rope (phase-scoped SBUF) ----
            with tc.tile_pool(name="pA", bufs=1) as pA, \
                 tc.tile_pool(name="psA", bufs=1, space="PSUM") as psA:
                # batched weight loads: DRAM [c, d] -> SBUF [c-sub(128), ci, d].
                # Issue order matters: the DMA engines drain in order, so wk
                # comes first (chunked between xt tiles), wv/wq mid-stream,
                # and wo not until phase B.
                wk_all = pA.tile([128, NCT, HD], f32r, tag="wk")
                nc.sync.dma_start(wk_all[:, 0:4, :], wk[0:512, :].bitcast(f32r)
                                  .rearrange("(c p) d -> p c d", p=128))
                wv_all = pA.tile([128, NCT, HD], f32r, tag="wv")
                wq_all = [pA.tile([128, NCT, HD], f32r, tag=f"wqh{h}",
                                  name=f"wq_all{h}") for h in range(GH)]

                def rope(ps, dst_ap, cs, sn):
                    """dst = ps*cos + swap64(ps)*sin_mod for token block tb."""
                    raw = pA.tile([128, TB], f32, tag="raw", bufs=3, name="raw")
                    nc.scalar.copy(raw[:], ps[:])
                    rot = pA.tile([128, TB], f32, tag="rot", bufs=6, name="rot")
                    nc.gpsimd.dma_start(rot[0:64, :], raw[64:128, :])
                    nc.gpsimd.dma_start(rot[64:128, :], raw[0:64, :])
                    m1 = pA.tile([128, TB], f32, tag="m1", bufs=6, name="m1")
                    nc.vector.tensor_tensor(m1[:], ps[:], cs[:], op=MUL)  # PSUM: DVE
                    nc.vector.tensor_tensor(rot[:], rot[:], sn[:], op=MUL)
                    nc.vector.tensor_tensor(dst_ap, m1[:], rot[:], op=ADD)

                for tb in range(NT):
                    cos_s = pA.tile([128, TB], f32, tag="cos", bufs=4, name="cos")
                    sin_s = pA.tile([128, TB], f32, tag="sin", bufs=4, name="sin")
                    if tb != 0:
                        # rope tables just ahead of the xt tiles
                        nc.sync.dma_start(cos_s[:], cos_d[:, TB * tb:TB * (tb + 1)])
                        nc.sync.dma_start(sin_s[:], sin_d[:, TB * tb:TB * (tb + 1)])
                    xt_t = []
                    for ci in range(NCT):
                        t = pA.tile([128, TB], f32r, tag="xt", bufs=31, name="xt")
                        nc.sync.dma_start(
                            t[:], xt[128 * ci:128 * (ci + 1),
                                     TB * tb:TB * (tb + 1)].bitcast(f32r))
                        xt_t.append(t)
                        if tb == 0 and ci % 4 == 3 and ci < 15:
                            c = ci // 4 + 1  # stream wk in behind the xt tiles
                            nc.sync.dma_start(
                                wk_all[:, 4 * c:4 * (c + 1), :],
                                wk[512 * c:512 * (c + 1), :].bitcast(f32r)
                                .rearrange("(c p) d -> p c d", p=128))
                        if tb == 0 and ci == 0:
                            nc.sync.dma_start(cos_s[:], cos_d[:, 0:TB])
                            nc.sync.dma_start(sin_s[:], sin_d[:, 0:TB])
                        if tb == 0 and ci == 1:
                            nc.sync.dma_start(ident_b[:], id_d[:])
                            warm = pA.tile([128, 1], f32, tag="warm")
                            nc.scalar.activation(warm[:], ident_b[:, 0:1], AF.Exp)
                        if tb == 0 and ci == 10:
                            # wv lands right as the V matmuls want it; K's last
                            # xt tiles shift later but K ends DMA-paced anyway
                            nc.sync.dma_start(wv_all[:], wv[:].bitcast(f32r)
                                              .rearrange("(c p) d -> p c d", p=128))
                    if tb == 0:
                        for h in range(GH):
                            nc.sync.dma_start(
                                wq_all[h][:], wq[:, HD * h:HD * (h + 1)]
                                .bitcast(f32r).rearrange("(c p) d -> p c d", p=128))
                    # K
                    ps_k = psA.tile([128, TB], f32, tag="pk")
                    for ci in range(NCT):
                        nc.tensor.matmul(ps_k[:], wk_all[:, ci, :], xt_t[ci][:],
                                         start=ci == 0, stop=ci == NCT - 1)
                    rope(ps_k, kt_rope[:, TB * tb:TB * (tb + 1)], cos_s, sin_s)

                    def emit_v():
                        ps_v = psA.tile([128, TB], f32, tag="pv")
                        for ci in range(NCT):
                            nc.tensor.matmul(ps_v[:], wv_all[:, ci, :],
                                             xt_t[ci][:],
                                             start=ci == 0, stop=ci == NCT - 1)
                        vt_sb = pA.tile([128, TB], bf16, tag="vts", bufs=3,
                                        name="vt_sb")
                        nc.scalar.copy(vt_sb[:], ps_v[:])
                        return vt_sb

                    def emit_vtrans(vt_sb, u):
                        ps_tr = psA.tile([128, 128], bf16, tag="ptr", bufs=1,
                                         name="ps_tr")
                        nc.tensor.transpose(ps_tr[:],
                                            vt_sb[:, 128 * u:128 * (u + 1)],
                                            ident_b[:])
                        # Act (not DVE): DVE's in-order queue sits behind rope
                        # m1 ops that can wait on the cos/sin loads
                        nc.scalar.copy(
                            v_all[:, 128 * (4 * tb + u):128 * (4 * tb + u + 1)],
                            ps_tr[:])

                    vt_sb = emit_v() if tb < NT - 1 else None
                    # Q heads; one V transpose is spread between each pair of
                    # head blocks so the single ptr bank's WAR (on the previous
                    # transpose's drain copy) never stalls the PE
                    for h in range(GH):
                        ps_q = psA.tile([128, TB], f32, tag="pq", bufs=5,
                                        name=f"ps_q{h}")
                        for ci in range(NCT):
                            nc.tensor.matmul(ps_q[:], wq_all[h][:, ci, :],
                                             xt_t[ci][:],
                                             start=ci == 0, stop=ci == NCT - 1)
                        if vt_sb is not None:
                            emit_vtrans(vt_sb, h)
                        rope(ps_q, qt_rope[h][:, TB * tb:TB * (tb + 1)], cos_s,
                             sin_s)
                    if vt_sb is None:
                        # last block: V after the Q heads, hiding the final
                        # rope chain's latency behind V's matmuls
                        vt_sb = emit_v()
                        for u in range(4):
                            emit_vtrans(vt_sb, u)

            # ---- phase B: attention (2-head passes) + partial o_proj ----
            with tc.tile_pool(name="pB", bufs=1) as pB, \
                 tc.tile_pool(name="psB", bufs=1, space="PSUM") as psB:
                # small constants are bf16 in DRAM: direct loads, no casts
                nc.sync.dma_start(cm_b[:], cm_d[:])
                nc.sync.dma_start(ones_b[:], on_d[:])
                # o_proj weights: first o_proj unit runs well into phase B, so
                # this load hides behind the first attention block
                wo_all = pB.tile([128, GH * HID], f32r, tag="wo")  # [j-sub, jh*2048+c]
                nc.sync.dma_start(wo_all[:].rearrange("p (h c) -> p h c", h=GH),
                                  wo[:].bitcast(f32r)
                                  .rearrange("(h p) c -> p h c", p=128))

                units = []   # pending o_proj 1-co unit closures

                def oproj_unit(j, at_j, co, dma_n):
                    """One 128-col chunk of q-block j's o_proj. dma_n: if set,
                    flush this ob group (last dma_n co tiles) to DRAM."""
                    gi = co % 2
                    ob = oproj_unit.ob
                    if gi == 0:
                        ob = oproj_unit.ob = pB.tile([128, 2, TB], f32,
                                                     tag="ob", bufs=3,
                                                     name="ob")
                    ps_p = psB.tile([128, TB], f32, tag="ps_d", bufs=2,
                                    name="ps_p")
                    for jh in range(GH):
                        nc.tensor.matmul(ps_p[:],
                                         wo_all[:, jh * HID + 128 * co:
                                                jh * HID + 128 * (co + 1)],
                                         at_j[jh][:], start=(jh == 0),
                                         stop=(jh == GH - 1))
                    if co % 2 == 1:
                        nc.scalar.copy(ob[:, gi, :], ps_p[:])
                    else:
                        nc.vector.tensor_copy(ob[:, gi, :], ps_p[:])
                    if dma_n:
                        if co < 4 and not with_rs:
                            dst = out_r[128 * (co + 1 - dma_n):128 * (co + 1),
                                        TB * j:TB * (j + 1)]
                        else:
                            dst = oT_part[128 * (co + 1 - dma_n):
                                          128 * (co + 1), TB * j:TB * (j + 1)]
                        nc.scalar.dma_start(
                            dst.rearrange("(u p) t -> p u t", p=128),
                            ob[:, gi + 1 - dma_n:gi + 1, :])
                oproj_unit.ob = None

                def make_units(j, at_j, tail=False):
                    # groups of 2 co per DMA; on the tail block split the last
                    # four into single-co DMAs so the final transfer is short
                    out = []
                    for co in range(NCT):
                        if tail and co >= NCT - 4:
                            dma_n = 1
                        else:
                            dma_n = 2 if co % 2 == 1 else 0
                        out.append((lambda jj, aa, cc, dd:
                                    lambda: oproj_unit(jj, aa, cc, dd))
                                   (j, at_j, co, dma_n))
                    return out

                order = [1, 2, 3, 0]
                for bi, j in enumerate(order):
                    last = bi == len(order) - 1
                    if causal:
                        tiles = [(i, 0) for i in range(4 * j)]
                        tiles += [(4 * j + m, min(128 * m, 256))
                                  for m in range(4)]
                    else:
                        tiles = [(i, 0) for i in range(NKT)]
                    last_i = tiles[-1][0]
                    n_iter = 2 * len(tiles)
                    # interleave cadence: previous block's units spread over
                    # this block's tile stream; on the last block hold 4 units
                    # back to cover the final normalize latency
                    reserve = 4 if last else 0
                    spread = max(0, len(units) - reserve)
                    credit = 0.0

                    at_j = [pB.tile([128, TB], f32r, tag=f"at{h}", bufs=2,
                                    name=f"at_s{h}") for h in range(GH)]
                    for p in range(2):
                        h0, h1 = 2 * p, 2 * p + 1
                        acc = pB.tile([128, 2 * TB], bf16, tag="accp", bufs=2,
                                      name="acc")
                        ps_o = {h: psB.tile([128, TB], f32, tag="po", bufs=2,
                                            name=f"ps_o{h}") for h in (h0, h1)}
                        for ti, (i, off) in enumerate(tiles):
                            w = TB - off
                            diag = causal and i >= 4 * j
                            m = i - 4 * j if diag else -1
                            ps2 = psB.tile([128, 2 * TB], f32, tag="ps_s",
                                           bufs=2, name="ps2")
                            for hh, h in enumerate((h0, h1)):
                                nc.tensor.matmul(
                                    ps2[:, TB * hh:TB * hh + w],
                                    kt_rope[:, 128 * i:128 * (i + 1)],
                                    qt_rope[h][:, TB * j + off:TB * (j + 1)],
                                    start=True, stop=True)
                            pt2 = pB.tile([128, 2 * TB], bf16, tag="pt",
                                          bufs=8, name="pt")
                            if w == TB:
                                nc.scalar.activation(pt2[:], ps2[:], AF.Exp)
                            else:
                                pr = pt2[:].rearrange("p (u q) -> p u q", u=2)
                                sr = ps2[:].rearrange("p (u q) -> p u q", u=2)
                                nc.scalar.activation(pr[:, :, 0:w],
                                                     sr[:, :, 0:w], AF.Exp)
                            if diag:
                                patt, pw = (1024, 256) if m == 3 else (0, TB)
                                cr = (cm_b[:, patt:patt + 2 * pw]
                                      .rearrange("p (u q) -> p u q", u=2))
                                pr = pt2[:].rearrange("p (u q) -> p u q", u=2)
                                nc.vector.tensor_tensor(
                                    pr[:, :, 0:w], pr[:, :, 0:w],
                                    cr[:, :, 0:w], op=MUL)
                            if ti == 0:
                                nc.vector.tensor_copy(acc[:], pt2[:])
                            else:
                                ar = acc[:].rearrange("p (u q) -> p u q", u=2)
                                pr = pt2[:].rearrange("p (u q) -> p u q", u=2)
                                nc.vector.tensor_tensor(
                                    ar[:, :, off:TB], ar[:, :, off:TB],
                                    pr[:, :, 0:w], op=ADD)
                            # PV (m=3 keep region is only the last 128 cols)
                            pv_off = 384 if m == 3 else off
                            for hh, h in enumerate((h0, h1)):
                                nc.tensor.matmul(
                                    ps_o[h][:, pv_off:TB],
                                    v_all[:, 128 * i:128 * (i + 1)],
                                    pt2[:, TB * hh + pv_off - off:
                                        TB * hh + TB - off],
                                    start=(ti == 0), stop=(i == last_i),
                                    skip_group_check=True)
                            credit += spread / n_iter
                            while credit >= 1.0 and units:
                                credit -= 1.0
                                units.pop(0)()
                        # pass end: paired denominator into the ps_s ring (so
                        # the ps_d ring stays free for o_proj units), then a
                        # couple of units cover the po-bank WAR into the next
                        # pass while the reciprocal + normalize chain runs
                        pdp = psB.tile([128, 2 * TB], f32, tag="ps_s", bufs=2,
                                       name="pdp")
                        for hh in range(2):
                            nc.tensor.matmul(pdp[:, TB * hh:TB * (hh + 1)],
                                             ones_b[:],
                                             acc[:, TB * hh:TB * (hh + 1)],
                                             start=True, stop=True)
                        if len(units) >= 2:
                            units.pop(0)(); units.pop(0)()
                        rec = pB.tile([128, 2 * TB], f32, tag="rec", bufs=2,
                                      name="rec")
                        nc.vector.reciprocal(rec[:], pdp[:])
                        for hh, h in enumerate((h0, h1)):
                            nc.vector.tensor_tensor(
                                at_j[h][:], ps_o[h][:],
                                rec[:, TB * hh:TB * (hh + 1)], op=MUL)
                    while units:
                        units.pop(0)()
                    units = make_units(j, at_j, tail=last)
                while units:
                    units.pop(0)()

            # ---- phase C: ReduceScatter partials, emit this core's slice ----
            if with_rs:
                nc.gpsimd.collective_compute(
                    "ReduceScatter", ADD,
                    replica_groups=[[0, 1, 2, 3], [4, 5, 6, 7]],
                    ins=[oT_part[:].opt()], outs=[oT_red[:].opt()],
                )
                nc.sync.dma_start(out_r[:], oT_red[:])

    nc.compile()
    return nc


def kernel(hidden_states, attention_mask, Wq, Wk, Wv, Wo, sin, cos):
    hidden_states = np.asarray(hidden_states, dtype=np.float32)
    attention_mask = np.asarray(attention_mask, dtype=np.float32)
    Wq, Wk, Wv, Wo = (np.ascontiguousarray(np.asarray(a, dtype=np.float32))
                      for a in (Wq, Wk, Wv, Wo))
    sin = np.asarray(sin, dtype=np.float32)
    cos = np.asarray(cos, dtype=np.float32)

    # classify the mask: causal (top-right strictly very-negative, elsewhere 0,
    # col 0 ignored since reference zeroes it) vs all-zeros (full attention)
    m0 = attention_mask[0, 0]
    iu = np.triu_indices(S, k=1)
    causal = bool((m0[iu] < -1e30).all() and
                  (m0[np.tril_indices(S, k=0)] == 0.0).all())
    if not causal:
        assert (attention_mask == 0).all(), "unsupported attention mask pattern"
    if causal:
        for b in range(1, B):
            assert np.array_equal(attention_mask[b, 0], m0), "mask differs per batch"

    key = causal
    if key not in _CACHE:
        _CACHE[key] = _build(causal)
    nc = _CACHE[key]

    import ml_dtypes
    nbf16 = ml_dtypes.bfloat16
    cos_t = np.ascontiguousarray(cos[:S].T)          # [128, S]
    sin_m = np.ascontiguousarray(sin[:S].T)
    sin_m[:64] *= -1.0
    # paired 0/1 causal keep-patterns (each repeated twice for head pairs):
    # patt0 = (q >= k) at cols 0:1024, patt1 = (q >= k + 128) at cols 1024:1536
    kl = np.arange(128)[:, None]
    ql = np.arange(512)[None, :]
    p0 = (ql >= kl).astype(np.float32)
    p1 = (ql[:, :256] >= kl + 128).astype(np.float32)
    cmask = np.concatenate([p0, p0, p1, p1], axis=1).astype(nbf16)

    in_maps = []
    for c in range(8):
        b, g = c // 4, c % 4
        in_maps.append({
            "xt": np.ascontiguousarray(hidden_states[b].T),
            "wq": np.ascontiguousarray(Wq[512 * g:512 * (g + 1), :].T),
            "wk": np.ascontiguousarray(Wk[128 * g:128 * (g + 1), :].T),
            "wv": np.ascontiguousarray(Wv[128 * g:128 * (g + 1), :].T),
            "wo": np.ascontiguousarray(Wo[:, 512 * g:512 * (g + 1)].T),
            "cos_t": cos_t, "sin_m": sin_m, "cmask": cmask,
            "ones_in": np.ones((128, 128), dtype=nbf16),
            "ident_in": np.eye(128, dtype=np.float32).astype(nbf16),
        })

    global _LAST_IN_MAPS, _LAST_RES
    _LAST_IN_MAPS = in_maps
    res = run_bass_kernel_spmd(nc, in_maps, core_ids=list(range(8)))
    _LAST_RES = res

    out = np.empty((B, S, HID), dtype=np.float32)
    for c in range(8):
        b, r = c // 4, c % 4
        out[b, :, TB * r:TB * (r + 1)] = res.results[c]["out_r"].T
    return out


if __name__ == "__main__":
    print("module loads ok")


# revision 24
# speedup vs baseline: 1.1699x; 1.0038x over previous
"""HF OpenMoe attention (B=2,S=2048,HID=2048,NH=16,NKV=4,HD=128) on 8 trn2 cores.

Sharding: core c -> (batch b=c//4, kv-group g=c%4). Each core computes Q/K/V
projections for its 4 query heads + 1 kv head, RoPE, causal flash attention in
S^T layout (scores transposed: [k, q], softmax over the partition dim via
ones-matmul), and its partial o_proj; a 4-way ReduceScatter sums the o_proj
partials, each core returning a 512-row slice of o^T for its batch.

Phase A streams x^T per token block with weights interleaved into the DMA
stream (wk chunked between xt tiles, wv mid-stream, wq after block 0), PE
chains emitted as operands land, rope on DVE/Act/Pool. Phase B runs causal
attention per q-block in two 2-head passes: score pairs land in a 2-bank PSUM
tile, one exp covers both heads, mask/acc run as paired 2x DVE ops, the
paired denominator reuses the score-PSUM ring, and o_proj of the previous
block is emitted in 1-co units between score tiles so the PE keeps running
through the Act-paced exp stream. The final block (j=0, shortest) holds a few
units in reserve to cover its normalize latency; tail o_proj DMAs are split
small and output DMAs ride the Act queue.
"""
import numpy as np
import concourse.bass as bass
import concourse.bacc as bacc
import concourse.tile as tile
import concourse.mybir as mybir
from concourse.bass_utils import run_bass_kernel_spmd

f32 = mybir.dt.float32
f32r = mybir.dt.float32r
bf16 = mybir.dt.bfloat16
AF = mybir.ActivationFunctionType
MUL = mybir.AluOpType.mult
ADD = mybir.AluOpType.add

B, S, HID = 2, 2048, 2048
NH, NKV, HD = 16, 4, 128
GH = NH // NKV          # query heads per core (4)
TB = 512                # token block (q block / projection block)
NT = S // TB            # 4 token blocks
NCT = HID // 128        # 16 contraction tiles
NKT = S // 128          # 16 key tiles

_CACHE = {}


def _build(causal: bool, with_rs: bool = True):
    nc = bacc.Bacc("TRN2", target_bir_lowering=False, debug=False, num_devices=8)
    xt = nc.dram_tensor("xt", [HID, S], f32, kind="ExternalInput").ap()
    wq = nc.dram_tensor("wq", [HID, GH * HD], f32, kind="ExternalInput").ap()
    wk = nc.dram_tensor("wk", [HID, HD], f32, kind="ExternalInput").ap()
    wv = nc.dram_tensor("wv", [HID, HD], f32, kind="ExternalInput").ap()
    wo = nc.dram_tensor("wo", [GH * HD, HID], f32, kind="ExternalInput").ap()
    cos_d = nc.dram_tensor("cos_t", [HD, S], f32, kind="ExternalInput").ap()
    sin_d = nc.dram_tensor("sin_m", [HD, S], f32, kind="ExternalInput").ap()
    cm_d = nc.dram_tensor("cmask", [128, 1536], bf16, kind="ExternalInput").ap()
    on_d = nc.dram_tensor("ones_in", [128, 128], bf16, kind="ExternalInput").ap()
    id_d = nc.dram_tensor("ident_in", [128, 128], bf16, kind="ExternalInput").ap()
    out_r = nc.dram_tensor("out_r", [TB, S], f32, kind="ExternalOutput").ap()

    with tile.TileContext(nc) as tc:
        with (
            tc.tile_pool(name="glob", bufs=1) as glob,
            tc.tile_pool(name="dram", bufs=1, space="DRAM") as dram,
        ):
            # ---- global resident stores ----
            kt_rope = glob.tile([128, S], f32r, tag="kt")          # roped K^T [d, k]
            v_all = glob.tile([128, S], bf16, tag="v")             # V natural (bf16)
            qt_rope = [glob.tile([128, S], f32r, tag=f"q{h}", name=f"qt_rope{h}")
                       for h in range(GH)]
            cm_b = glob.tile([128, 1536], bf16, tag="cmb")         # paired 0/1 masks
            ones_b = glob.tile([128, 128], bf16, tag="onesb")
            ident_b = glob.tile([128, 128], bf16, tag="identb")

            oT_part = dram.tile([HID, S], f32)                     # o^T partial
            oT_red = dram.tile([TB, S], f32)

            # ---- phase A: projections + rope (phase-scoped SBUF) ----
            with tc.tile_pool(name="pA", bufs=1) as pA, \
                 tc.tile_pool(name="psA", bufs=1, space="PSUM") as psA:
                # batched weight loads: DRAM [c, d] -> SBUF [c-sub(128), ci, d].
                # Issue order matters: the DMA engines drain in order, so wk
                # comes first (chunked between xt tiles), wv/wq mid-stream,
                # and wo not until phase B.
                wk_all = pA.tile([128, NCT, HD], f32r, tag="wk")
                nc.sync.dma_start(wk_all[:, 0:4, :], wk[0:512, :].bitcast(f32r)
                                  .rearrange("(c p) d -> p c d", p=128))
                wv_all = pA.tile([128, NCT, HD], f32r, tag="wv")
                wq_all = [pA.tile([128, NCT, HD], f32r, tag=f"wqh{h}",
                                  name=f"wq_all{h}") for h in range(GH)]

                def rope(ps, dst_ap, cs, sn):
                    """dst = ps*cos + swap64(ps)*sin_mod for token block tb."""
                    raw = pA.tile([128, TB], f32, tag="raw", bufs=3, name="raw")
                    nc.scalar.copy(raw[:], ps[:])
                    rot = pA.tile([128, TB], f32, tag="rot", bufs=6, name="rot")
                    nc.gpsimd.dma_start(rot[0:64, :], raw[64:128, :])
                    nc.gpsimd.dma_start(rot[64:128, :], raw[0:64, :])
                    m1 = pA.tile([128, TB], f32, tag="m1", bufs=6, name="m1")
                    nc.vector.tensor_tensor(m1[:], ps[:], cs[:], op=MUL)  # PSUM: DVE
                    nc.vector.tensor_tensor(rot[:], rot[:], sn[:], op=MUL)
                    nc.vector.tensor_tensor(dst_ap, m1[:], rot[:], op=ADD)

                for tb in range(NT):
                    cos_s = pA.tile([128, TB], f32, tag="cos", bufs=4, name="cos")
                    sin_s = pA.tile([128, TB], f32, tag="sin", bufs=4, name="sin")
                    if tb != 0:
                        # rope tables just ahead of the xt tiles
                        nc.sync.dma_start(cos_s[:], cos_d[:, TB * tb:TB * (tb + 1)])
                        nc.sync.dma_start(sin_s[:], sin_d[:, TB * tb:TB * (tb + 1)])
                    xt_t = []
                    for ci in range(NCT):
                        t = pA.tile([128, TB], f32r, tag="xt", bufs=31, name="xt")
                        nc.sync.dma_start(
                            t[:], xt[128 * ci:128 * (ci + 1),
                                     TB * tb:TB * (tb + 1)].bitcast(f32r))
                        xt_t.append(t)
                        if tb == 0 and ci % 4 == 3 and ci < 15:
                            c = ci // 4 + 1  # stream wk in behind the xt tiles
                            nc.sync.dma_start(
                                wk_all[:, 4 * c:4 * (c + 1), :],
                                wk[512 * c:512 * (c + 1), :].bitcast(f32r)
                                .rearrange("(c p) d -> p c d", p=128))
                        if tb == 0 and ci == 0:
                            nc.sync.dma_start(cos_s[:], cos_d[:, 0:TB])
                            nc.sync.dma_start(sin_s[:], sin_d[:, 0:TB])
                        if tb == 0 and ci == 1:
                            nc.sync.dma_start(ident_b[:], id_d[:])
                            warm = pA.tile([128, 1], f32, tag="warm")
                            nc.scalar.activation(warm[:], ident_b[:, 0:1], AF.Exp)
                        if tb == 0 and ci == 10:
                            # wv lands right as the V matmuls want it; K's last
                            # xt tiles shift later but K ends DMA-paced anyway
                            nc.sync.dma_start(wv_all[:], wv[:].bitcast(f32r)
                                              .rearrange("(c p) d -> p c d", p=128))
                    if tb == 0:
                        for h in range(GH):
                            nc.sync.dma_start(
                                wq_all[h][:], wq[:, HD * h:HD * (h + 1)]
                                .bitcast(f32r).rearrange("(c p) d -> p c d", p=128))
                    # K
                    ps_k = psA.tile([128, TB], f32, tag="pk")
                    for ci in range(NCT):
                        nc.tensor.matmul(ps_k[:], wk_all[:, ci, :], xt_t[ci][:],
                                         start=ci == 0, stop=ci == NCT - 1)
                    rope(ps_k, kt_rope[:, TB * tb:TB * (tb + 1)], cos_s, sin_s)

                    def emit_v():
                        ps_v = psA.tile([128, TB], f32, tag="pv")
                        for ci in range(NCT):
                            nc.tensor.matmul(ps_v[:], wv_all[:, ci, :],
                                             xt_t[ci][:],
                                             start=ci == 0, stop=ci == NCT - 1)
                        vt_sb = pA.tile([128, TB], bf16, tag="vts", bufs=3,
                                        name="vt_sb")
                        nc.scalar.copy(vt_sb[:], ps_v[:])
                        return vt_sb

                    def emit_vtrans(vt_sb, u):
                        ps_tr = psA.tile([128, 128], bf16, tag="ptr", bufs=1,
                                         name="ps_tr")
                        nc.tensor.transpose(ps_tr[:],
                                            vt_sb[:, 128 * u:128 * (u + 1)],
                                            ident_b[:])
                        # Act (not DVE): DVE's in-order queue sits behind rope
                        # m1 ops that can wait on the cos/sin loads
                        nc.scalar.copy(
                            v_all[:, 128 * (4 * tb + u):128 * (4 * tb + u + 1)],
                            ps_tr[:])

                    vt_sb = emit_v() if tb < NT - 1 else None
                    # Q heads; one V transpose is spread between each pair of
                    # head blocks so the single ptr bank's WAR (on the previous
                    # transpose's drain copy) never stalls the PE
                    for h in range(GH):
                        ps_q = psA.tile([128, TB], f32, tag="pq", bufs=5,
                                        name=f"ps_q{h}")
                        for ci in range(NCT):
                            nc.tensor.matmul(ps_q[:], wq_all[h][:, ci, :],
                                             xt_t[ci][:],
                                             start=ci == 0, stop=ci == NCT - 1)
                        if vt_sb is not None:
                            emit_vtrans(vt_sb, h)
                        rope(ps_q, qt_rope[h][:, TB * tb:TB * (tb + 1)], cos_s,
                             sin_s)
                    if vt_sb is None:
                        # last block: V after the Q heads, hiding the final
                        # rope chain's latency behind V's matmuls
                        vt_sb = emit_v()
                        for u in range(4):
                            emit_vtrans(vt_sb, u)

            # ---- phase B: attention (2-head passes) + partial o_proj ----
            with tc.tile_pool(name="pB", bufs=1) as pB, \
                 tc.tile_pool(name="psB", bufs=1, space="PSUM") as psB:
                # small constants are bf16 in DRAM: direct loads, no casts
                nc.sync.dma_start(cm_b[:], cm_d[:])
                nc.sync.dma_start(ones_b[:], on_d[:])
                # o_proj weights: first o_proj unit runs well into phase B, so
                # this load hides behind the first attention block
                wo_all = pB.tile([128, GH * HID], f32r, tag="wo")  # [j-sub, jh*2048+c]
                nc.sync.dma_start(wo_all[:].rearrange("p (h c) -> p h c", h=GH),
                                  wo[:].bitcast(f32r)
                                  .rearrange("(h p) c -> p h c", p=128))

                units = []   # pending o_proj 1-co unit closures

                def oproj_unit(j, at_j, co, dma_n):
                    """One 128-col chunk of q-block j's o_proj. dma_n=2: flush
                    the 2-co ob group; dma_n=1: tail unit, DMA straight from
                    PSUM (skips the ob staging copy to shorten the tail)."""
                    gi = co % 2
                    ob = oproj_unit.ob
                    if gi == 0 and dma_n != 1:
                        ob = oproj_unit.ob = pB.tile([128, 2, TB], f32,
                                                     tag="ob", bufs=3,
                                                     name="ob")
                    ps_p = psB.tile([128, TB], f32, tag="ps_d", bufs=2,
                                    name="ps_p")
                    for jh in range(GH):
                        nc.tensor.matmul(ps_p[:],
                                         wo_all[:, jh * HID + 128 * co:
                                                jh * HID + 128 * (co + 1)],
                                         at_j[jh][:], start=(jh == 0),
                                         stop=(jh == GH - 1))
                    if co < 4 and not with_rs:
                        full = out_r
                    else:
                        full = oT_part
                    if dma_n == 1:
                        dst = full[128 * co:128 * (co + 1),
                                   TB * j:TB * (j + 1)]
                        obt = pB.tile([128, TB], f32, tag="obt", bufs=2,
                                      name="obt")
                        if co % 2:
                            nc.scalar.copy(obt[:], ps_p[:])
                            nc.scalar.dma_start(dst, obt[:])
                        else:
                            nc.vector.tensor_copy(obt[:], ps_p[:])
                            nc.sync.dma_start(dst, obt[:])
                        return
                    if co % 2 == 1:
                        nc.scalar.copy(ob[:, gi, :], ps_p[:])
                    else:
                        nc.vector.tensor_copy(ob[:, gi, :], ps_p[:])
                    if dma_n:
                        dst = full[128 * (co + 1 - dma_n):128 * (co + 1),
                                   TB * j:TB * (j + 1)]
                        nc.scalar.dma_start(
                            dst.rearrange("(u p) t -> p u t", p=128),
                            ob[:, gi + 1 - dma_n:gi + 1, :])
                oproj_unit.ob = None

                def make_units(j, at_j, tail=False):
                    # groups of 2 co per DMA; on the tail block the last four
                    # co go straight from PSUM in single-co DMAs
                    out = []
                    for co in range(NCT):
                        if tail and co >= NCT - 4:
                            dma_n = 1
                        else:
                            dma_n = 2 if co % 2 == 1 else 0
                        out.append((lambda jj, aa, cc, dd:
                                    lambda: oproj_unit(jj, aa, cc, dd))
                                   (j, at_j, co, dma_n))
                    return out

                order = [1, 2, 3, 0]
                for bi, j in enumerate(order):
                    last = bi == len(order) - 1
                    if causal:
                        tiles = [(i, 0) for i in range(4 * j)]
                        tiles += [(4 * j + m, min(128 * m, 256))
                                  for m in range(4)]
                    else:
                        tiles = [(i, 0) for i in range(NKT)]
                    last_i = tiles[-1][0]
                    n_iter = 2 * len(tiles)
                    # interleave cadence: previous block's units spread over
                    # this block's tile stream, holding 2 back per pass end
                    # (they cover the denominator-reciprocal PSUM-slot WAR)
                    reserve = 4
                    spread = max(0, len(units) - reserve)
                    credit = 0.0

                    at_j = [pB.tile([128, TB], f32r, tag=f"at{h}", bufs=2,
                                    name=f"at_s{h}") for h in range(GH)]
                    for p in range(2):
                        h0, h1 = 2 * p, 2 * p + 1
                        acc = pB.tile([128, 2 * TB], bf16, tag="accp", bufs=2,
                                      name="acc")
                        ps_o = {h: psB.tile([128, TB], f32, tag="po", bufs=2,
                                            name=f"ps_o{h}") for h in (h0, h1)}
                        for ti, (i, off) in enumerate(tiles):
                            w = TB - off
                            diag = causal and i >= 4 * j
                            m = i - 4 * j if diag else -1
                            ps2 = psB.tile([128, 2 * TB], f32, tag="ps_s",
                                           bufs=2, name="ps2")
                            for hh, h in enumerate((h0, h1)):
                                nc.tensor.matmul(
                                    ps2[:, TB * hh:TB * hh + w],
                                    kt_rope[:, 128 * i:128 * (i + 1)],
                                    qt_rope[h][:, TB * j + off:TB * (j + 1)],
                                    start=True, stop=True)
                            pt2 = pB.tile([128, 2 * TB], bf16, tag="pt",
                                          bufs=8, name="pt")
                            if w == TB:
                                nc.scalar.activation(pt2[:], ps2[:], AF.Exp)
                            else:
                                pr = pt2[:].rearrange("p (u q) -> p u q", u=2)
                                sr = ps2[:].rearrange("p (u q) -> p u q", u=2)
                                nc.scalar.activation(pr[:, :, 0:w],
                                                     sr[:, :, 0:w], AF.Exp)
                            if diag:
                                patt, pw = (1024, 256) if m == 3 else (0, TB)
                                cr = (cm_b[:, patt:patt + 2 * pw]
                                      .rearrange("p (u q) -> p u q", u=2))
                                pr = pt2[:].rearrange("p (u q) -> p u q", u=2)
                                nc.vector.tensor_tensor(
                                    pr[:, :, 0:w], pr[:, :, 0:w],
                                    cr[:, :, 0:w], op=MUL)
                            if ti == 0:
                                nc.vector.tensor_copy(acc[:], pt2[:])
                            else:
                                ar = acc[:].rearrange("p (u q) -> p u q", u=2)
                                pr = pt2[:].rearrange("p (u q) -> p u q", u=2)
                                nc.vector.tensor_tensor(
                                    ar[:, :, off:TB], ar[:, :, off:TB],
                                    pr[:, :, 0:w], op=ADD)
                            # PV (m=3 keep region is only the last 128 cols)
                            pv_off = 384 if m == 3 else off
                            for hh, h in enumerate((h0, h1)):
                                nc.tensor.matmul(
                                    ps_o[h][:, pv_off:TB],
                                    v_all[:, 128 * i:128 * (i + 1)],
                                    pt2[:, TB * hh + pv_off - off:
                                        TB * hh + TB - off],
                                    start=(ti == 0), stop=(i == last_i),
                                    skip_group_check=True)
                            credit += spread / n_iter
                            while credit >= 1.0 and units:
                                credit -= 1.0
                                units.pop(0)()
                        # pass end: paired denominator into the ps_s ring (so
                        # the ps_d ring stays free for o_proj units), then a
                        # couple of units cover the po-bank WAR into the next
                        # pass while the reciprocal + normalize chain runs
                        pdp = psB.tile([128, 2 * TB], f32, tag="ps_s", bufs=2,
                                       name="pdp")
                        for hh in range(2):
                            nc.tensor.matmul(pdp[:, TB * hh:TB * (hh + 1)],
                                             ones_b[:],
                                             acc[:, TB * hh:TB * (hh + 1)],
                                             start=True, stop=True)
                        if len(units) >= 2:
                            units.pop(0)(); units.pop(0)()
                        rec = pB.tile([128, 2 * TB], f32, tag="rec", bufs=2,
                                      name="rec")
                        nc.vector.reciprocal(rec[:], pdp[:])
                        for hh, h in enumerate((h0, h1)):
                            nc.vector.tensor_tensor(
                                at_j[h][:], ps_o[h][:],
                                rec[:, TB * hh:TB * (hh + 1)], op=MUL)
                    while units:
                        units.pop(0)()
                    units = make_units(j, at_j, tail=last)
                while units:
                    units.pop(0)()

            # ---- phase C: ReduceScatter partials, emit this core's slice ----
            if with_rs:
                nc.gpsimd.collective_compute(
                    "ReduceScatter", ADD,
                    replica_groups=[[0, 1, 2, 3], [4, 5, 6, 7]],
                    ins=[oT_part[:].opt()], outs=[oT_red[:].opt()],
                )
                nc.sync.dma_start(out_r[:], oT_red[:])

    nc.compile()
    return nc


def kernel(hidden_states, attention_mask, Wq, Wk, Wv, Wo, sin, cos):
    hidden_states = np.asarray(hidden_states, dtype=np.float32)
    attention_mask = np.asarray(attention_mask, dtype=np.float32)
    Wq, Wk, Wv, Wo = (np.ascontiguousarray(np.asarray(a, dtype=np.float32))
                      for a in (Wq, Wk, Wv, Wo))
    sin = np.asarray(sin, dtype=np.float32)
    cos = np.asarray(cos, dtype=np.float32)

    # classify the mask: causal (top-right strictly very-negative, elsewhere 0,
    # col 0 ignored since reference zeroes it) vs all-zeros (full attention)
    m0 = attention_mask[0, 0]
    iu = np.triu_indices(S, k=1)
    causal = bool((m0[iu] < -1e30).all() and
                  (m0[np.tril_indices(S, k=0)] == 0.0).all())
    if not causal:
        assert (attention_mask == 0).all(), "unsupported attention mask pattern"
    if causal:
        for b in range(1, B):
            assert np.array_equal(attention_mask[b, 0], m0), "mask differs per batch"

    key = causal
    if key not in _CACHE:
        _CACHE[key] = _build(causal)
    nc = _CACHE[key]

    import ml_dtypes
    nbf16 = ml_dtypes.bfloat16
    cos_t = np.ascontiguousarray(cos[:S].T)          # [128, S]
    sin_m = np.ascontiguousarray(sin[:S].T)
    sin_m[:64] *= -1.0
    # paired 0/1 causal keep-patterns (each repeated twice for head pairs):
    # patt0 = (q >= k) at cols 0:1024, patt1 = (q >= k + 128) at cols 1024:1536
    kl = np.arange(128)[:, None]
    ql = np.arange(512)[None, :]
    p0 = (ql >= kl).astype(np.float32)
    p1 = (ql[:, :256] >= kl + 128).astype(np.float32)
    cmask = np.concatenate([p0, p0, p1, p1], axis=1).astype(nbf16)

    in_maps = []
    for c in range(8):
        b, g = c // 4, c % 4
        in_maps.append({
            "xt": np.ascontiguousarray(hidden_states[b].T),
            "wq": np.ascontiguousarray(Wq[512 * g:512 * (g + 1), :].T),
            "wk": np.ascontiguousarray(Wk[128 * g:128 * (g + 1), :].T),
            "wv": np.ascontiguousarray(Wv[128 * g:128 * (g + 1), :].T),
            "wo": np.ascontiguousarray(Wo[:, 512 * g:512 * (g + 1)].T),
            "cos_t": cos_t, "sin_m": sin_m, "cmask": cmask,
            "ones_in": np.ones((128, 128), dtype=nbf16),
            "ident_in": np.eye(128, dtype=np.float32).astype(nbf16),
        })

    global _LAST_IN_MAPS, _LAST_RES
    _LAST_IN_MAPS = in_maps
    res = run_bass_kernel_spmd(nc, in_maps, core_ids=list(range(8)))
    _LAST_RES = res

    out = np.empty((B, S, HID), dtype=np.float32)
    for c in range(8):
        b, r = c // 4, c % 4
        out[b, :, TB * r:TB * (r + 1)] = res.results[c]["out_r"].T
    return out


if __name__ == "__main__":
    print("module loads ok")


# revision 30
# speedup vs baseline: 1.1981x; 1.0242x over previous
"""HF OpenMoe attention (B=2,S=2048,HID=2048,NH=16,NKV=4,HD=128) on 8 trn2 cores.

Sharding: core c -> (batch b=c//4, kv-group g=c%4). Each core computes Q/K/V
projections for its 4 query heads + 1 kv head, RoPE, causal flash attention in
S^T layout (scores transposed: [k, q], softmax over the partition dim via
ones-matmul), and its partial o_proj; a 4-way ReduceScatter sums the o_proj
partials, each core returning a 512-row slice of o^T for its batch.

Phase A streams x^T per token block with weights interleaved into the DMA
stream (wk chunked between xt tiles, wv mid-stream, wq after block 0), PE
chains emitted as operands land, rope on DVE/Act/Pool. Phase B runs causal
attention per q-block in two 2-head passes: score pairs land in a 2-bank PSUM
tile, one exp covers both heads, mask/acc run as paired 2x DVE ops, the
paired denominator reuses the score-PSUM ring, and o_proj of the previous
block is emitted in 1-co units between score tiles so the PE keeps running
through the Act-paced exp stream. The final block (j=0, shortest) holds a few
units in reserve to cover its normalize latency; tail o_proj DMAs are split
small and output DMAs ride the Act queue.
"""
import numpy as np
import concourse.bass as bass
import concourse.bacc as bacc
import concourse.tile as tile
import concourse.mybir as mybir
from concourse.bass_utils import run_bass_kernel_spmd

f32 = mybir.dt.float32
f32r = mybir.dt.float32r
bf16 = mybir.dt.bfloat16
AF = mybir.ActivationFunctionType
MUL = mybir.AluOpType.mult
ADD = mybir.AluOpType.add

B, S, HID = 2, 2048, 2048
NH, NKV, HD = 16, 4, 128
GH = NH // NKV          # query heads per core (4)
TB = 512                # token block (q block / projection block)
NT = S // TB            # 4 token blocks
NCT = HID // 128        # 16 contraction tiles
NKT = S // 128          # 16 key tiles

_CACHE = {}


def _build(causal: bool, with_rs: bool = True):
    nc = bacc.Bacc("TRN2", target_bir_lowering=False, debug=False, num_devices=8)
    xt = nc.dram_tensor("xt", [HID, S], f32, kind="ExternalInput").ap()
    wq = nc.dram_tensor("wq", [HID, GH * HD], f32, kind="ExternalInput").ap()
    wk = nc.dram_tensor("wk", [HID, HD], f32, kind="ExternalInput").ap()
    wv = nc.dram_tensor("wv", [HID, HD], f32, kind="ExternalInput").ap()
    wo = nc.dram_tensor("wo", [GH * HD, HID], f32, kind="ExternalInput").ap()
    cos_d = nc.dram_tensor("cos_t", [HD, S], f32, kind="ExternalInput").ap()
    sin_d = nc.dram_tensor("sin_m", [HD, S], f32, kind="ExternalInput").ap()
    cm_d = nc.dram_tensor("cmask", [128, 1536], bf16, kind="ExternalInput").ap()
    on_d = nc.dram_tensor("ones_in", [128, 128], bf16, kind="ExternalInput").ap()
    id_d = nc.dram_tensor("ident_in", [128, 128], bf16, kind="ExternalInput").ap()
    out_r = nc.dram_tensor("out_r", [TB, S], f32, kind="ExternalOutput").ap()

    with tile.TileContext(nc) as tc:
        with (
            tc.tile_pool(name="glob", bufs=1) as glob,
            tc.tile_pool(name="dram", bufs=1, space="DRAM") as dram,
        ):
            # ---- global resident stores ----
            kt_rope = glob.tile([128, S], f32r, tag="kt")          # roped K^T [d, k]
            v_all = glob.tile([128, S], bf16, tag="v")             # V natural (bf16)
            qt_rope = [glob.tile([128, S], f32r, tag=f"q{h}", name=f"qt_rope{h}")
                       for h in range(GH)]
            cm_b = glob.tile([128, 1536], bf16, tag="cmb")         # paired 0/1 masks
            ones_b = glob.tile([128, 128], bf16, tag="onesb")
            ident_b = glob.tile([128, 128], bf16, tag="identb")

            # partials are bf16: halves the o_proj output + collective bytes
            # (the final out_r slice stays f32)
            oT_part = dram.tile([HID, S], bf16)                    # o^T partial
            oT_red = dram.tile([TB, S], bf16)

            # ---- phase A: projections + rope (phase-scoped SBUF) ----
            with tc.tile_pool(name="pA", bufs=1) as pA, \
                 tc.tile_pool(name="psA", bufs=1, space="PSUM") as psA:
                # batched weight loads: DRAM [c, d] -> SBUF [c-sub(128), ci, d].
                # Issue order matters: the DMA engines drain in order, so wk
                # comes first (chunked between xt tiles), wv/wq mid-stream,
                # and wo not until phase B.
                wk_all = pA.tile([128, NCT, HD], f32r, tag="wk")
                nc.sync.dma_start(wk_all[:, 0:4, :], wk[0:512, :].bitcast(f32r)
                                  .rearrange("(c p) d -> p c d", p=128))
                wv_all = pA.tile([128, NCT, HD], f32r, tag="wv")
                wq_all = [pA.tile([128, NCT, HD], f32r, tag=f"wqh{h}",
                                  name=f"wq_all{h}") for h in range(GH)]

                def rope(ps, dst_ap, cs, sn):
                    """dst = ps*cos + swap64(ps)*sin_mod for token block tb."""
                    raw = pA.tile([128, TB], f32, tag="raw", bufs=3, name="raw")
                    nc.scalar.copy(raw[:], ps[:])
                    rot = pA.tile([128, TB], f32, tag="rot", bufs=6, name="rot")
                    nc.gpsimd.dma_start(rot[0:64, :], raw[64:128, :])
                    nc.gpsimd.dma_start(rot[64:128, :], raw[0:64, :])
                    m1 = pA.tile([128, TB], f32, tag="m1", bufs=6, name="m1")
                    nc.vector.tensor_tensor(m1[:], ps[:], cs[:], op=MUL)  # PSUM: DVE
                    nc.vector.tensor_tensor(rot[:], rot[:], sn[:], op=MUL)
                    nc.vector.tensor_tensor(dst_ap, m1[:], rot[:], op=ADD)

                for tb in range(NT):
                    cos_s = pA.tile([128, TB], f32, tag="cos", bufs=4, name="cos")
                    sin_s = pA.tile([128, TB], f32, tag="sin", bufs=4, name="sin")
                    if tb != 0:
                        # rope tables just ahead of the xt tiles
                        nc.sync.dma_start(cos_s[:], cos_d[:, TB * tb:TB * (tb + 1)])
                        nc.sync.dma_start(sin_s[:], sin_d[:, TB * tb:TB * (tb + 1)])
                    xt_t = []
                    for ci in range(NCT):
                        t = pA.tile([128, TB], f32r, tag="xt", bufs=31, name="xt")
                        nc.sync.dma_start(
                            t[:], xt[128 * ci:128 * (ci + 1),
                                     TB * tb:TB * (tb + 1)].bitcast(f32r))
                        xt_t.append(t)
                        if tb == 0 and ci % 4 == 3 and ci < 15:
                            c = ci // 4 + 1  # stream wk in behind the xt tiles
                            nc.sync.dma_start(
                                wk_all[:, 4 * c:4 * (c + 1), :],
                                wk[512 * c:512 * (c + 1), :].bitcast(f32r)
                                .rearrange("(c p) d -> p c d", p=128))
                        if tb == 0 and ci == 0:
                            nc.sync.dma_start(cos_s[:], cos_d[:, 0:TB])
                            nc.sync.dma_start(sin_s[:], sin_d[:, 0:TB])
                        if tb == 0 and ci == 1:
                            nc.sync.dma_start(ident_b[:], id_d[:])
                            warm = pA.tile([128, 1], f32, tag="warm")
                            nc.scalar.activation(warm[:], ident_b[:, 0:1], AF.Exp)
                        if tb == 0 and ci == 10:
                            # wv lands right as the V matmuls want it; K's last
                            # xt tiles shift later but K ends DMA-paced anyway
                            nc.sync.dma_start(wv_all[:], wv[:].bitcast(f32r)
                                              .rearrange("(c p) d -> p c d", p=128))
                    if tb == 0:
                        for h in range(GH):
                            nc.sync.dma_start(
                                wq_all[h][:], wq[:, HD * h:HD * (h + 1)]
                                .bitcast(f32r).rearrange("(c p) d -> p c d", p=128))
                    # K
                    ps_k = psA.tile([128, TB], f32, tag="pk")
                    for ci in range(NCT):
                        nc.tensor.matmul(ps_k[:], wk_all[:, ci, :], xt_t[ci][:],
                                         start=ci == 0, stop=ci == NCT - 1)
                    rope(ps_k, kt_rope[:, TB * tb:TB * (tb + 1)], cos_s, sin_s)

                    def emit_v():
                        ps_v = psA.tile([128, TB], f32, tag="pv")
                        for ci in range(NCT):
                            nc.tensor.matmul(ps_v[:], wv_all[:, ci, :],
                                             xt_t[ci][:],
                                             start=ci == 0, stop=ci == NCT - 1)
                        vt_sb = pA.tile([128, TB], bf16, tag="vts", bufs=3,
                                        name="vt_sb")
                        nc.scalar.copy(vt_sb[:], ps_v[:])
                        return vt_sb

                    def emit_vtrans(vt_sb, u):
                        ps_tr = psA.tile([128, 128], bf16, tag="ptr", bufs=1,
                                         name="ps_tr")
                        nc.tensor.transpose(ps_tr[:],
                                            vt_sb[:, 128 * u:128 * (u + 1)],
                                            ident_b[:])
                        # Act (not DVE): DVE's in-order queue sits behind rope
                        # m1 ops that can wait on the cos/sin loads
                        nc.scalar.copy(
                            v_all[:, 128 * (4 * tb + u):128 * (4 * tb + u + 1)],
                            ps_tr[:])

                    vt_sb = emit_v() if tb < NT - 1 else None
                    # Q heads; one V transpose is spread between each pair of
                    # head blocks so the single ptr bank's WAR (on the previous
                    # transpose's drain copy) never stalls the PE
                    for h in range(GH):
                        ps_q = psA.tile([128, TB], f32, tag="pq", bufs=5,
                                        name=f"ps_q{h}")
                        for ci in range(NCT):
                            nc.tensor.matmul(ps_q[:], wq_all[h][:, ci, :],
                                             xt_t[ci][:],
                                             start=ci == 0, stop=ci == NCT - 1)
                        if vt_sb is not None:
                            emit_vtrans(vt_sb, h)
                        rope(ps_q, qt_rope[h][:, TB * tb:TB * (tb + 1)], cos_s,
                             sin_s)
                    if vt_sb is None:
                        # last block: V after the Q heads, hiding the final
                        # rope chain's latency behind V's matmuls
                        vt_sb = emit_v()
                        for u in range(4):
                            emit_vtrans(vt_sb, u)

            # ---- phase B: attention (2-head passes) + partial o_proj ----
            with tc.tile_pool(name="pB", bufs=1) as pB, \
                 tc.tile_pool(name="psB", bufs=1, space="PSUM") as psB:
                # small constants are bf16 in DRAM: direct loads, no casts
                nc.sync.dma_start(cm_b[:], cm_d[:])
                nc.sync.dma_start(ones_b[:], on_d[:])
                # o_proj weights: first o_proj unit runs well into phase B, so
                # this load hides behind the first attention block
                wo_all = pB.tile([128, GH * HID], f32r, tag="wo")  # [j-sub, jh*2048+c]
                nc.sync.dma_start(wo_all[:].rearrange("p (h c) -> p h c", h=GH),
                                  wo[:].bitcast(f32r)
                                  .rearrange("(h p) c -> p h c", p=128))

                units = []   # pending o_proj 1-co unit closures

                def oproj_unit(j, at_j, co, dma_n):
                    """One 128-col chunk of q-block j's o_proj. dma_n=2: flush
                    the 2-co ob group; dma_n=1: tail unit, DMA straight from
                    PSUM (skips the ob staging copy to shorten the tail)."""
                    gi = co % 2
                    ob = oproj_unit.ob
                    if gi == 0 and dma_n != 1:
                        ob = oproj_unit.ob = pB.tile([128, 2, TB], bf16,
                                                     tag="ob", bufs=3,
                                                     name="ob")
                    ps_p = psB.tile([128, TB], f32, tag="ps_d", bufs=2,
                                    name="ps_p")
                    for jh in range(GH):
                        nc.tensor.matmul(ps_p[:],
                                         wo_all[:, jh * HID + 128 * co:
                                                jh * HID + 128 * (co + 1)],
                                         at_j[jh][:], start=(jh == 0),
                                         stop=(jh == GH - 1))
                    if dma_n == 1:
                        if co < 4 and not with_rs:
                            dst = out_r[128 * co:128 * (co + 1),
                                        TB * j:TB * (j + 1)]
                            obt = pB.tile([128, TB], f32, tag="obt", bufs=2,
                                          name="obt")
                        else:
                            dst = oT_part[128 * co:128 * (co + 1),
                                          TB * j:TB * (j + 1)]
                            obt = pB.tile([128, TB], bf16, tag="obtb", bufs=2,
                                          name="obtb")
                        if co % 2:
                            nc.scalar.copy(obt[:], ps_p[:])
                            nc.scalar.dma_start(dst, obt[:])
                        else:
                            nc.vector.tensor_copy(obt[:], ps_p[:])
                            nc.sync.dma_start(dst, obt[:])
                        return
                    if co % 2 == 1:
                        nc.scalar.copy(ob[:, gi, :], ps_p[:])
                    else:
                        nc.vector.tensor_copy(ob[:, gi, :], ps_p[:])
                    if dma_n:
                        dst = oT_part[128 * (co + 1 - dma_n):128 * (co + 1),
                                      TB * j:TB * (j + 1)]
                        nc.scalar.dma_start(
                            dst.rearrange("(u p) t -> p u t", p=128),
                            ob[:, gi + 1 - dma_n:gi + 1, :])
                oproj_unit.ob = None

                def make_units(j, at_j, tail=False):
                    # groups of 2 co per DMA into bf16 oT_part; single-co DMAs
                    # for the out_r slice (no-RS build) and the tail block's
                    # last four co (short final transfers)
                    out = []
                    for co in range(NCT):
                        if (co < 4 and not with_rs) or (tail and co >= NCT - 4):
                            dma_n = 1
                        else:
                            dma_n = 2 if co % 2 == 1 else 0
                        out.append((lambda jj, aa, cc, dd:
                                    lambda: oproj_unit(jj, aa, cc, dd))
                                   (j, at_j, co, dma_n))
                    return out

                order = [1, 2, 3, 0]
                for bi, j in enumerate(order):
                    last = bi == len(order) - 1
                    if causal:
                        tiles = [(i, 0) for i in range(4 * j)]
                        tiles += [(4 * j + m, min(128 * m, 256))
                                  for m in range(4)]
                    else:
                        tiles = [(i, 0) for i in range(NKT)]
                    last_i = tiles[-1][0]
                    n_iter = 2 * len(tiles)
                    # interleave cadence: previous block's units spread over
                    # this block's tile stream, holding 2 back per pass end
                    # (they cover the denominator-reciprocal PSUM-slot WAR)
                    res_units = units[max(0, len(units) - 4):]
                    units = units[:max(0, len(units) - 4)]
                    spread = len(units)
                    credit = 0.0

                    at_j = [pB.tile([128, TB], f32r, tag=f"at{h}", bufs=2,
                                    name=f"at_s{h}") for h in range(GH)]
                    for p in range(2):
                        h0, h1 = 2 * p, 2 * p + 1
                        acc = pB.tile([128, 2 * TB], bf16, tag="accp", bufs=2,
                                      name="acc")
                        ps_o = {h: psB.tile([128, TB], f32, tag="po", bufs=2,
                                            name=f"ps_o{h}") for h in (h0, h1)}
                        for ti, (i, off) in enumerate(tiles):
                            w = TB - off
                            diag = causal and i >= 4 * j
                            m = i - 4 * j if diag else -1
                            ps2 = psB.tile([128, 2 * TB], f32, tag="ps_s",
                                           bufs=2, name="ps2")
                            for hh, h in enumerate((h0, h1)):
                                nc.tensor.matmul(
                                    ps2[:, TB * hh:TB * hh + w],
                                    kt_rope[:, 128 * i:128 * (i + 1)],
                                    qt_rope[h][:, TB * j + off:TB * (j + 1)],
                                    start=True, stop=True)
                            pt2 = pB.tile([128, 2 * TB], bf16, tag="pt",
                                          bufs=8, name="pt")
                            if w == TB:
                                nc.scalar.activation(pt2[:], ps2[:], AF.Exp)
                            else:
                                pr = pt2[:].rearrange("p (u q) -> p u q", u=2)
                                sr = ps2[:].rearrange("p (u q) -> p u q", u=2)
                                nc.scalar.activation(pr[:, :, 0:w],
                                                     sr[:, :, 0:w], AF.Exp)
                            if diag:
                                patt, pw = (1024, 256) if m == 3 else (0, TB)
                                cr = (cm_b[:, patt:patt + 2 * pw]
                                      .rearrange("p (u q) -> p u q", u=2))
                                pr = pt2[:].rearrange("p (u q) -> p u q", u=2)
                                nc.vector.tensor_tensor(
                                    pr[:, :, 0:w], pr[:, :, 0:w],
                                    cr[:, :, 0:w], op=MUL)
                            if ti == 0:
                                nc.vector.tensor_copy(acc[:], pt2[:])
                            else:
                                ar = acc[:].rearrange("p (u q) -> p u q", u=2)
                                pr = pt2[:].rearrange("p (u q) -> p u q", u=2)
                                nc.vector.tensor_tensor(
                                    ar[:, :, off:TB], ar[:, :, off:TB],
                                    pr[:, :, 0:w], op=ADD)
                            # PV (m=3 keep region is only the last 128 cols)
                            pv_off = 384 if m == 3 else off
                            for hh, h in enumerate((h0, h1)):
                                nc.tensor.matmul(
                                    ps_o[h][:, pv_off:TB],
                                    v_all[:, 128 * i:128 * (i + 1)],
                                    pt2[:, TB * hh + pv_off - off:
                                        TB * hh + TB - off],
                                    start=(ti == 0), stop=(i == last_i),
                                    skip_group_check=True)
                            credit += spread / n_iter
                            while credit >= 1.0 and units:
                                credit -= 1.0
                                units.pop(0)()
                        # pass end: paired denominator into the ps_s ring (so
                        # the ps_d ring stays free for o_proj units), then a
                        # couple of units cover the po-bank WAR into the next
                        # pass while the reciprocal + normalize chain runs
                        pdp = psB.tile([128, 2 * TB], f32, tag="ps_s", bufs=2,
                                       name="pdp")
                        for hh in range(2):
                            nc.tensor.matmul(pdp[:, TB * hh:TB * (hh + 1)],
                                             ones_b[:],
                                             acc[:, TB * hh:TB * (hh + 1)],
                                             start=True, stop=True)
                        for _ in range(min(2, len(res_units))):
                            res_units.pop(0)()
                        rec = pB.tile([128, 2 * TB], f32, tag="rec", bufs=2,
                                      name="rec")
                        nc.vector.reciprocal(rec[:], pdp[:])
                        for hh, h in enumerate((h0, h1)):
                            nc.vector.tensor_tensor(
                                at_j[h][:], ps_o[h][:],
                                rec[:, TB * hh:TB * (hh + 1)], op=MUL)
                    for u in units + res_units:
                        u()
                    units = make_units(j, at_j, tail=last)
                for u in units:
                    u()

            # ---- phase C: ReduceScatter partials, emit this core's slice ----
            if with_rs:
                nc.gpsimd.collective_compute(
                    "ReduceScatter", ADD,
                    replica_groups=[[0, 1, 2, 3], [4, 5, 6, 7]],
                    ins=[oT_part[:].opt()], outs=[oT_red[:].opt()],
                )
                # bf16 partial sum -> f32 output slice
                with tc.tile_pool(name="pC", bufs=1) as pC:
                    for u in range(4):
                        tb16 = pC.tile([128, S], bf16, tag="c16", bufs=2,
                                       name="c16")
                        nc.sync.dma_start(tb16[:],
                                          oT_red[128 * u:128 * (u + 1), :])
                        t32 = pC.tile([128, S], f32, tag="c32", bufs=2,
                                      name="c32")
                        nc.scalar.copy(t32[:], tb16[:])
                        nc.sync.dma_start(out_r[128 * u:128 * (u + 1), :],
                                          t32[:])

    nc.compile()
    return nc


def kernel(hidden_states, attention_mask, Wq, Wk, Wv, Wo, sin, cos):
    hidden_states = np.asarray(hidden_states, dtype=np.float32)
    attention_mask = np.asarray(attention_mask, dtype=np.float32)
    Wq, Wk, Wv, Wo = (np.ascontiguousarray(np.asarray(a, dtype=np.float32))
                      for a in (Wq, Wk, Wv, Wo))
    sin = np.asarray(sin, dtype=np.float32)
    cos = np.asarray(cos, dtype=np.float32)

    # classify the mask: causal (top-right strictly very-negative, elsewhere 0,
    # col 0 ignored since reference zeroes it) vs all-zeros (full attention)
    m0 = attention_mask[0, 0]
    iu = np.triu_indices(S, k=1)
    causal = bool((m0[iu] < -1e30).all() and
                  (m0[np.tril_indices(S, k=0)] == 0.0).all())
    if not causal:
        assert (attention_mask == 0).all(), "unsupported attention mask pattern"
    if causal:
        for b in range(1, B):
            assert np.array_equal(attention_mask[b, 0], m0), "mask differs per batch"

    key = causal
    if key not in _CACHE:
        _CACHE[key] = _build(causal)
    nc = _CACHE[key]

    import ml_dtypes
    nbf16 = ml_dtypes.bfloat16
    cos_t = np.ascontiguousarray(cos[:S].T)          # [128, S]
    sin_m = np.ascontiguousarray(sin[:S].T)
    sin_m[:64] *= -1.0
    # paired 0/1 causal keep-patterns (each repeated twice for head pairs):
    # patt0 = (q >= k) at cols 0:1024, patt1 = (q >= k + 128) at cols 1024:1536
    kl = np.arange(128)[:, None]
    ql = np.arange(512)[None, :]
    p0 = (ql >= kl).astype(np.float32)
    p1 = (ql[:, :256] >= kl + 128).astype(np.float32)
    cmask = np.concatenate([p0, p0, p1, p1], axis=1).astype(nbf16)

    in_maps = []
    for c in range(8):
        b, g = c // 4, c % 4
        in_maps.append({
            "xt": np.ascontiguousarray(hidden_states[b].T),
            "wq": np.ascontiguousarray(Wq[512 * g:512 * (g + 1), :].T),
            "wk": np.ascontiguousarray(Wk[128 * g:128 * (g + 1), :].T),
            "wv": np.ascontiguousarray(Wv[128 * g:128 * (g + 1), :].T),
            "wo": np.ascontiguousarray(Wo[:, 512 * g:512 * (g + 1)].T),
            "cos_t": cos_t, "sin_m": sin_m, "cmask": cmask,
            "ones_in": np.ones((128, 128), dtype=nbf16),
            "ident_in": np.eye(128, dtype=np.float32).astype(nbf16),
        })

    global _LAST_IN_MAPS, _LAST_RES
    _LAST_IN_MAPS = in_maps
    res = run_bass_kernel_spmd(nc, in_maps, core_ids=list(range(8)))
    _LAST_RES = res

    out = np.empty((B, S, HID), dtype=np.float32)
    for c in range(8):
        b, r = c // 4, c % 4
        out[b, :, TB * r:TB * (r + 1)] = res.results[c]["out_r"].T
    return out


if __name__ == "__main__":
    print("module loads ok")


# revision 34
# speedup vs baseline: 1.2020x; 1.0032x over previous
"""HF OpenMoe attention (B=2,S=2048,HID=2048,NH=16,NKV=4,HD=128) on 8 trn2 cores.

Sharding: core c -> (batch b=c//4, kv-group g=c%4). Each core computes Q/K/V
projections for its 4 query heads + 1 kv head, RoPE, causal flash attention in
S^T layout (scores transposed: [k, q], softmax over the partition dim via
ones-matmul), and its partial o_proj; a 4-way ReduceScatter sums the o_proj
partials, each core returning a 512-row slice of o^T for its batch.

Phase A streams x^T per token block with weights interleaved into the DMA
stream (wk chunked between xt tiles, wv mid-stream, wq after block 0), PE
chains emitted as operands land, rope on DVE/Act/Pool. Phase B runs causal
attention per q-block in two 2-head passes: score pairs land in a 2-bank PSUM
tile, one exp covers both heads, mask/acc run as paired 2x DVE ops, the
paired denominator reuses the score-PSUM ring, and o_proj of the previous
block is emitted in 1-co units between score tiles so the PE keeps running
through the Act-paced exp stream. The final block (j=0, shortest) holds a few
units in reserve to cover its normalize latency; tail o_proj DMAs are split
small and output DMAs ride the Act queue.
"""
import numpy as np
import concourse.bass as bass
import concourse.bacc as bacc
import concourse.tile as tile
import concourse.mybir as mybir
from concourse.bass_utils import run_bass_kernel_spmd

f32 = mybir.dt.float32
f32r = mybir.dt.float32r
bf16 = mybir.dt.bfloat16
AF = mybir.ActivationFunctionType
MUL = mybir.AluOpType.mult
ADD = mybir.AluOpType.add

B, S, HID = 2, 2048, 2048
NH, NKV, HD = 16, 4, 128
GH = NH // NKV          # query heads per core (4)
TB = 512                # token block (q block / projection block)
NT = S // TB            # 4 token blocks
NCT = HID // 128        # 16 contraction tiles
NKT = S // 128          # 16 key tiles

_CACHE = {}


def _build(causal: bool, with_rs: bool = True):
    nc = bacc.Bacc("TRN2", target_bir_lowering=False, debug=False, num_devices=8)
    xt = nc.dram_tensor("xt", [HID, S], f32, kind="ExternalInput").ap()
    wq = nc.dram_tensor("wq", [HID, GH * HD], f32, kind="ExternalInput").ap()
    wk = nc.dram_tensor("wk", [HID, HD], f32, kind="ExternalInput").ap()
    wv = nc.dram_tensor("wv", [HID, HD], f32, kind="ExternalInput").ap()
    wo = nc.dram_tensor("wo", [GH * HD, HID], f32, kind="ExternalInput").ap()
    cos_d = nc.dram_tensor("cos_t", [HD, S], f32, kind="ExternalInput").ap()
    sin_d = nc.dram_tensor("sin_m", [HD, S], f32, kind="ExternalInput").ap()
    cm_d = nc.dram_tensor("cmask", [128, 1536], bf16, kind="ExternalInput").ap()
    on_d = nc.dram_tensor("ones_in", [128, 128], bf16, kind="ExternalInput").ap()
    id_d = nc.dram_tensor("ident_in", [128, 128], bf16, kind="ExternalInput").ap()
    out_r = nc.dram_tensor("out_r", [TB, S], f32, kind="ExternalOutput").ap()

    with tile.TileContext(nc) as tc:
        with (
            tc.tile_pool(name="glob", bufs=1) as glob,
            tc.tile_pool(name="dram", bufs=1, space="DRAM") as dram,
        ):
            # ---- global resident stores ----
            kt_rope = glob.tile([128, S], f32r, tag="kt")          # roped K^T [d, k]
            v_all = glob.tile([128, S], bf16, tag="v")             # V natural (bf16)
            qt_rope = [glob.tile([128, S], f32r, tag=f"q{h}", name=f"qt_rope{h}")
                       for h in range(GH)]
            cm_b = glob.tile([128, 1536], bf16, tag="cmb")         # paired 0/1 masks
            ones_b = glob.tile([128, 128], bf16, tag="onesb")
            ident_b = glob.tile([128, 128], bf16, tag="identb")

            # partials are bf16: halves the o_proj output + collective bytes
            # (the final out_r slice stays f32)
            oT_part = dram.tile([HID, S], bf16)                    # o^T partial
            oT_red = dram.tile([TB, S], bf16)

            # ---- phase A: projections + rope (phase-scoped SBUF) ----
            with tc.tile_pool(name="pA", bufs=1) as pA, \
                 tc.tile_pool(name="psA", bufs=1, space="PSUM") as psA:
                # batched weight loads: DRAM [c, d] -> SBUF [c-sub(128), ci, d].
                # Issue order matters: the DMA engines drain in order, so wk
                # comes first (chunked between xt tiles), wv/wq mid-stream,
                # and wo not until phase B.
                wk_all = pA.tile([128, NCT, HD], f32r, tag="wk")
                nc.sync.dma_start(wk_all[:, 0:1, :], wk[0:128, :].bitcast(f32r)
                                  .rearrange("(c p) d -> p c d", p=128))
                wv_all = pA.tile([128, NCT, HD], f32r, tag="wv")
                wq_all = [pA.tile([128, NCT, HD], f32r, tag=f"wqh{h}",
                                  name=f"wq_all{h}") for h in range(GH)]

                def rope(ps, dst_ap, cs, sn):
                    """dst = ps*cos + swap64(ps)*sin_mod for token block tb."""
                    raw = pA.tile([128, TB], f32, tag="raw", bufs=3, name="raw")
                    nc.scalar.copy(raw[:], ps[:])
                    rot = pA.tile([128, TB], f32, tag="rot", bufs=6, name="rot")
                    nc.gpsimd.dma_start(rot[0:64, :], raw[64:128, :])
                    nc.gpsimd.dma_start(rot[64:128, :], raw[0:64, :])
                    m1 = pA.tile([128, TB], f32, tag="m1", bufs=6, name="m1")
                    nc.vector.tensor_tensor(m1[:], ps[:], cs[:], op=MUL)  # PSUM: DVE
                    nc.vector.tensor_tensor(rot[:], rot[:], sn[:], op=MUL)
                    nc.vector.tensor_tensor(dst_ap, m1[:], rot[:], op=ADD)

                for tb in range(NT):
                    cos_s = pA.tile([128, TB], f32, tag="cos", bufs=4, name="cos")
                    sin_s = pA.tile([128, TB], f32, tag="sin", bufs=4, name="sin")
                    if tb != 0:
                        # rope tables just ahead of the xt tiles
                        nc.sync.dma_start(cos_s[:], cos_d[:, TB * tb:TB * (tb + 1)])
                        nc.sync.dma_start(sin_s[:], sin_d[:, TB * tb:TB * (tb + 1)])
                    xt_t = []
                    for ci in range(NCT):
                        t = pA.tile([128, TB], f32r, tag="xt", bufs=31, name="xt")
                        nc.sync.dma_start(
                            t[:], xt[128 * ci:128 * (ci + 1),
                                     TB * tb:TB * (tb + 1)].bitcast(f32r))
                        xt_t.append(t)
                        if tb == 0 and ci == 0:
                            nc.sync.dma_start(
                                wk_all[:, 1:4, :], wk[128:512, :].bitcast(f32r)
                                .rearrange("(c p) d -> p c d", p=128))
                        if tb == 0 and ci % 4 == 3 and ci < 15:
                            c = ci // 4 + 1  # stream wk in behind the xt tiles
                            nc.sync.dma_start(
                                wk_all[:, 4 * c:4 * (c + 1), :],
                                wk[512 * c:512 * (c + 1), :].bitcast(f32r)
                                .rearrange("(c p) d -> p c d", p=128))
                        if tb == 0 and ci == 0:
                            nc.sync.dma_start(cos_s[:], cos_d[:, 0:TB])
                            nc.sync.dma_start(sin_s[:], sin_d[:, 0:TB])
                        if tb == 0 and ci == 1:
                            nc.sync.dma_start(ident_b[:], id_d[:])
                            warm = pA.tile([128, 1], f32, tag="warm")
                            nc.scalar.activation(warm[:], ident_b[:, 0:1], AF.Exp)
                        if tb == 0 and ci == 6:
                            # wv early: V proj fills the K-chain's DMA lag
                            nc.sync.dma_start(wv_all[:], wv[:].bitcast(f32r)
                                              .rearrange("(c p) d -> p c d", p=128))
                        if tb == 0 and ci == 10:
                            # first q head's weights ahead of the rest
                            nc.sync.dma_start(
                                wq_all[0][:], wq[:, 0:HD]
                                .bitcast(f32r).rearrange("(c p) d -> p c d", p=128))
                    if tb == 0:
                        for h in range(1, GH):
                            nc.sync.dma_start(
                                wq_all[h][:], wq[:, HD * h:HD * (h + 1)]
                                .bitcast(f32r).rearrange("(c p) d -> p c d", p=128))
                    # K
                    ps_k = psA.tile([128, TB], f32, tag="pk")
                    for ci in range(NCT):
                        nc.tensor.matmul(ps_k[:], wk_all[:, ci, :], xt_t[ci][:],
                                         start=ci == 0, stop=ci == NCT - 1)
                    rope(ps_k, kt_rope[:, TB * tb:TB * (tb + 1)], cos_s, sin_s)

                    def emit_v():
                        ps_v = psA.tile([128, TB], f32, tag="pv")
                        for ci in range(NCT):
                            nc.tensor.matmul(ps_v[:], wv_all[:, ci, :],
                                             xt_t[ci][:],
                                             start=ci == 0, stop=ci == NCT - 1)
                        vt_sb = pA.tile([128, TB], bf16, tag="vts", bufs=3,
                                        name="vt_sb")
                        nc.scalar.copy(vt_sb[:], ps_v[:])
                        return vt_sb

                    def emit_vtrans(vt_sb, u):
                        ps_tr = psA.tile([128, 128], bf16, tag="ptr", bufs=1,
                                         name="ps_tr")
                        nc.tensor.transpose(ps_tr[:],
                                            vt_sb[:, 128 * u:128 * (u + 1)],
                                            ident_b[:])
                        # Act (not DVE): DVE's in-order queue sits behind rope
                        # m1 ops that can wait on the cos/sin loads
                        nc.scalar.copy(
                            v_all[:, 128 * (4 * tb + u):128 * (4 * tb + u + 1)],
                            ps_tr[:])

                    vt_sb = emit_v() if tb < NT - 1 else None
                    # Q heads; one V transpose is spread between each pair of
                    # head blocks so the single ptr bank's WAR (on the previous
                    # transpose's drain copy) never stalls the PE
                    for h in range(GH):
                        ps_q = psA.tile([128, TB], f32, tag="pq", bufs=5,
                                        name=f"ps_q{h}")
                        for ci in range(NCT):
                            nc.tensor.matmul(ps_q[:], wq_all[h][:, ci, :],
                                             xt_t[ci][:],
                                             start=ci == 0, stop=ci == NCT - 1)
                        if vt_sb is not None:
                            emit_vtrans(vt_sb, h)
                        rope(ps_q, qt_rope[h][:, TB * tb:TB * (tb + 1)], cos_s,
                             sin_s)
                    if vt_sb is None:
                        # last block: V after the Q heads, hiding the final
                        # rope chain's latency behind V's matmuls
                        vt_sb = emit_v()
                        for u in range(4):
                            emit_vtrans(vt_sb, u)

            # ---- phase B: attention (2-head passes) + partial o_proj ----
            with tc.tile_pool(name="pB", bufs=1) as pB, \
                 tc.tile_pool(name="psB", bufs=1, space="PSUM") as psB:
                # small constants are bf16 in DRAM: direct loads, no casts
                nc.sync.dma_start(cm_b[:], cm_d[:])
                nc.sync.dma_start(ones_b[:], on_d[:])
                # o_proj weights: first o_proj unit runs well into phase B, so
                # this load hides behind the first attention block
                wo_all = pB.tile([128, GH * HID], f32r, tag="wo")  # [j-sub, jh*2048+c]
                nc.sync.dma_start(wo_all[:].rearrange("p (h c) -> p h c", h=GH),
                                  wo[:].bitcast(f32r)
                                  .rearrange("(h p) c -> p h c", p=128))

                units = []   # pending o_proj 1-co unit closures

                def oproj_unit(j, at_j, co, dma_n):
                    """One 128-col chunk of q-block j's o_proj. dma_n=2: flush
                    the 2-co ob group; dma_n=1: tail unit, DMA straight from
                    PSUM (skips the ob staging copy to shorten the tail)."""
                    gi = co % 2
                    ob = oproj_unit.ob
                    if gi == 0 and dma_n != 1:
                        ob = oproj_unit.ob = pB.tile([128, 2, TB], bf16,
                                                     tag="ob", bufs=4,
                                                     name="ob")
                    ps_p = psB.tile([128, TB], f32, tag="ps_d", bufs=2,
                                    name="ps_p")
                    for jh in range(GH):
                        nc.tensor.matmul(ps_p[:],
                                         wo_all[:, jh * HID + 128 * co:
                                                jh * HID + 128 * (co + 1)],
                                         at_j[jh][:], start=(jh == 0),
                                         stop=(jh == GH - 1))
                    if dma_n == 1:
                        if co < 4 and not with_rs:
                            dst = out_r[128 * co:128 * (co + 1),
                                        TB * j:TB * (j + 1)]
                            obt = pB.tile([128, TB], f32, tag="obt", bufs=2,
                                          name="obt")
                        else:
                            dst = oT_part[128 * co:128 * (co + 1),
                                          TB * j:TB * (j + 1)]
                            obt = pB.tile([128, TB], bf16, tag="obtb", bufs=2,
                                          name="obtb")
                        if co % 2:
                            nc.scalar.copy(obt[:], ps_p[:])
                            nc.scalar.dma_start(dst, obt[:])
                        else:
                            nc.vector.tensor_copy(obt[:], ps_p[:])
                            nc.sync.dma_start(dst, obt[:])
                        return
                    if co % 2 == 1:
                        nc.scalar.copy(ob[:, gi, :], ps_p[:])
                    else:
                        nc.vector.tensor_copy(ob[:, gi, :], ps_p[:])
                    if dma_n:
                        dst = oT_part[128 * (co + 1 - dma_n):128 * (co + 1),
                                      TB * j:TB * (j + 1)]
                        q = nc.scalar if (co // 2) % 2 else nc.sync
                        q.dma_start(
                            dst.rearrange("(u p) t -> p u t", p=128),
                            ob[:, gi + 1 - dma_n:gi + 1, :])
                oproj_unit.ob = None

                def make_units(j, at_j, tail=False):
                    # groups of 2 co per DMA into bf16 oT_part; single-co DMAs
                    # for the out_r slice (no-RS build) and the tail block's
                    # last four co (short final transfers)
                    out = []
                    for co in range(NCT):
                        if (co < 4 and not with_rs) or (tail and co >= NCT - 4):
                            dma_n = 1
                        else:
                            dma_n = 2 if co % 2 == 1 else 0
                        out.append((lambda jj, aa, cc, dd:
                                    lambda: oproj_unit(jj, aa, cc, dd))
                                   (j, at_j, co, dma_n))
                    return out

                order = [1, 2, 3, 0]
                for bi, j in enumerate(order):
                    last = bi == len(order) - 1
                    if causal:
                        tiles = [(i, 0) for i in range(4 * j)]
                        tiles += [(4 * j + m, min(128 * m, 256))
                                  for m in range(4)]
                    else:
                        tiles = [(i, 0) for i in range(NKT)]
                    last_i = tiles[-1][0]
                    n_iter = 2 * len(tiles)
                    # interleave cadence: previous block's units spread over
                    # this block's tile stream, holding 2 back per pass end
                    # (they cover the denominator-reciprocal PSUM-slot WAR)
                    res_units = units[max(0, len(units) - 4):]
                    units = units[:max(0, len(units) - 4)]
                    spread = len(units)
                    credit = 0.0

                    at_j = [pB.tile([128, TB], f32r, tag=f"at{h}", bufs=2,
                                    name=f"at_s{h}") for h in range(GH)]
                    for p in range(2):
                        h0, h1 = 2 * p, 2 * p + 1
                        acc = pB.tile([128, 2 * TB], bf16, tag="accp", bufs=2,
                                      name="acc")
                        ps_o = {h: psB.tile([128, TB], f32, tag="po", bufs=2,
                                            name=f"ps_o{h}") for h in (h0, h1)}
                        for ti, (i, off) in enumerate(tiles):
                            w = TB - off
                            diag = causal and i >= 4 * j
                            m = i - 4 * j if diag else -1
                            ps2 = psB.tile([128, 2 * TB], f32, tag="ps_s",
                                           bufs=2, name="ps2")
                            for hh, h in enumerate((h0, h1)):
                                nc.tensor.matmul(
                                    ps2[:, TB * hh:TB * hh + w],
                                    kt_rope[:, 128 * i:128 * (i + 1)],
                                    qt_rope[h][:, TB * j + off:TB * (j + 1)],
                                    start=True, stop=True)
                            pt2 = pB.tile([128, 2 * TB], bf16, tag="pt",
                                          bufs=8, name="pt")
                            if w == TB:
                                nc.scalar.activation(pt2[:], ps2[:], AF.Exp)
                            else:
                                pr = pt2[:].rearrange("p (u q) -> p u q", u=2)
                                sr = ps2[:].rearrange("p (u q) -> p u q", u=2)
                                nc.scalar.activation(pr[:, :, 0:w],
                                                     sr[:, :, 0:w], AF.Exp)
                            if diag:
                                patt, pw = (1024, 256) if m == 3 else (0, TB)
                                cr = (cm_b[:, patt:patt + 2 * pw]
                                      .rearrange("p (u q) -> p u q", u=2))
                                pr = pt2[:].rearrange("p (u q) -> p u q", u=2)
                                nc.vector.tensor_tensor(
                                    pr[:, :, 0:w], pr[:, :, 0:w],
                                    cr[:, :, 0:w], op=MUL)
                            if ti == 0:
                                nc.vector.tensor_copy(acc[:], pt2[:])
                            else:
                                ar = acc[:].rearrange("p (u q) -> p u q", u=2)
                                pr = pt2[:].rearrange("p (u q) -> p u q", u=2)
                                nc.vector.tensor_tensor(
                                    ar[:, :, off:TB], ar[:, :, off:TB],
                                    pr[:, :, 0:w], op=ADD)
                            # PV (m=3 keep region is only the last 128 cols)
                            pv_off = 384 if m == 3 else off
                            for hh, h in enumerate((h0, h1)):
                                nc.tensor.matmul(
                                    ps_o[h][:, pv_off:TB],
                                    v_all[:, 128 * i:128 * (i + 1)],
                                    pt2[:, TB * hh + pv_off - off:
                                        TB * hh + TB - off],
                                    start=(ti == 0), stop=(i == last_i),
                                    skip_group_check=True)
                            credit += spread / n_iter
                            while credit >= 1.0 and units:
                                credit -= 1.0
                                units.pop(0)()
                        # pass end: paired denominator into the ps_s ring (so
                        # the ps_d ring stays free for o_proj units), then a
                        # couple of units cover the po-bank WAR into the next
                        # pass while the reciprocal + normalize chain runs
                        pdp = psB.tile([128, 2 * TB], f32, tag="ps_s", bufs=2,
                                       name="pdp")
                        for hh in range(2):
                            nc.tensor.matmul(pdp[:, TB * hh:TB * (hh + 1)],
                                             ones_b[:],
                                             acc[:, TB * hh:TB * (hh + 1)],
                                             start=True, stop=True)
                        for _ in range(min(2, len(res_units))):
                            res_units.pop(0)()
                        rec = pB.tile([128, 2 * TB], f32, tag="rec", bufs=2,
                                      name="rec")
                        nc.vector.reciprocal(rec[:], pdp[:])
                        for hh, h in enumerate((h0, h1)):
                            nc.vector.tensor_tensor(
                                at_j[h][:], ps_o[h][:],
                                rec[:, TB * hh:TB * (hh + 1)], op=MUL)
                    for u in units + res_units:
                        u()
                    units = make_units(j, at_j, tail=last)
                for u in units:
                    u()

            # ---- phase C: ReduceScatter partials, emit this core's slice ----
            if with_rs:
                nc.gpsimd.collective_compute(
                    "ReduceScatter", ADD,
                    replica_groups=[[0, 1, 2, 3], [4, 5, 6, 7]],
                    ins=[oT_part[:].opt()], outs=[oT_red[:].opt()],
                )
                # bf16 partial sum -> f32 output slice
                with tc.tile_pool(name="pC", bufs=1) as pC:
                    for u in range(4):
                        tb16 = pC.tile([128, S], bf16, tag="c16", bufs=2,
                                       name="c16")
                        nc.sync.dma_start(tb16[:],
                                          oT_red[128 * u:128 * (u + 1), :])
                        t32 = pC.tile([128, S], f32, tag="c32", bufs=2,
                                      name="c32")
                        nc.scalar.copy(t32[:], tb16[:])
                        nc.sync.dma_start(out_r[128 * u:128 * (u + 1), :],
                                          t32[:])

    nc.compile()
    return nc


def kernel(hidden_states, attention_mask, Wq, Wk, Wv, Wo, sin, cos):
    hidden_states = np.asarray(hidden_states, dtype=np.float32)
    attention_mask = np.asarray(attention_mask, dtype=np.float32)
    Wq, Wk, Wv, Wo = (np.ascontiguousarray(np.asarray(a, dtype=np.float32))
                      for a in (Wq, Wk, Wv, Wo))
    sin = np.asarray(sin, dtype=np.float32)
    cos = np.asarray(cos, dtype=np.float32)

    # classify the mask: causal (top-right strictly very-negative, elsewhere 0,
    # col 0 ignored since reference zeroes it) vs all-zeros (full attention)
    m0 = attention_mask[0, 0]
    iu = np.triu_indices(S, k=1)
    causal = bool((m0[iu] < -1e30).all() and
                  (m0[np.tril_indices(S, k=0)] == 0.0).all())
    if not causal:
        assert (attention_mask == 0).all(), "unsupported attention mask pattern"
    if causal:
        for b in range(1, B):
            assert np.array_equal(attention_mask[b, 0], m0), "mask differs per batch"

    key = causal
    if key not in _CACHE:
        _CACHE[key] = _build(causal)
    nc = _CACHE[key]

    import ml_dtypes
    nbf16 = ml_dtypes.bfloat16
    cos_t = np.ascontiguousarray(cos[:S].T)          # [128, S]
    sin_m = np.ascontiguousarray(sin[:S].T)
    sin_m[:64] *= -1.0
    # paired 0/1 causal keep-patterns (each repeated twice for head pairs):
    # patt0 = (q >= k) at cols 0:1024, patt1 = (q >= k + 128) at cols 1024:1536
    kl = np.arange(128)[:, None]
    ql = np.arange(512)[None, :]
    p0 = (ql >= kl).astype(np.float32)
    p1 = (ql[:, :256] >= kl + 128).astype(np.float32)
    cmask = np.concatenate([p0, p0, p1, p1], axis=1).astype(nbf16)

    in_maps = []
    for c in range(8):
        b, g = c // 4, c % 4
        in_maps.append({
            "xt": np.ascontiguousarray(hidden_states[b].T),
            "wq": np.ascontiguousarray(Wq[512 * g:512 * (g + 1), :].T),
            "wk": np.ascontiguousarray(Wk[128 * g:128 * (g + 1), :].T),
            "wv": np.ascontiguousarray(Wv[128 * g:128 * (g + 1), :].T),
            "wo": np.ascontiguousarray(Wo[:, 512 * g:512 * (g + 1)].T),
            "cos_t": cos_t, "sin_m": sin_m, "cmask": cmask,
            "ones_in": np.ones((128, 128), dtype=nbf16),
            "ident_in": np.eye(128, dtype=np.float32).astype(nbf16),
        })

    global _LAST_IN_MAPS, _LAST_RES
    _LAST_IN_MAPS = in_maps
    res = run_bass_kernel_spmd(nc, in_maps, core_ids=list(range(8)))
    _LAST_RES = res

    out = np.empty((B, S, HID), dtype=np.float32)
    for c in range(8):
        b, r = c // 4, c % 4
        out[b, :, TB * r:TB * (r + 1)] = res.results[c]["out_r"].T
    return out


if __name__ == "__main__":
    print("module loads ok")
